# revision 2
# baseline (speedup 1.0000x reference)
"""Trainium2 Bass kernel: batched serial-chain forward kinematics (fp16).

Problem: nn_DifferentiableRobotModel — q [262144, 12] joint angles,
per-link constant transforms. Output [B, 12, 12] = per link
(flattened 3x3 rot, 3 trans).

Math (per batch element b, per link i, sequential over i):
    Rj_i = A_i + sin(q_i) * B_i + cos(q_i) * C_i     (3x3)
    R_i  = R_{i-1} @ Rj_i        (R_{-1} = I)
    t_i  = t_{i-1} + R_{i-1} @ tf_i   (t_{-1} = 0)
with host-precomputed per-link constants:
    A_i = Rf_i + Rf_i@K_i@K_i ;  B_i = Rf_i@K_i ;  C_i = -Rf_i@K_i@K_i
    (K = skew(axis)), tf_i = trans_fixed_i.

Device strategy: pure data parallel over 8 cores (batch split). Compute
in fp16, batch-innermost layout [..., E=256]. DVE runs the serial chain
(augmented [Rj|tf] product per link, 2x_1P mode on every op) plus the
rj builds for links 0-5; the otherwise-idle GpSimd (Pool) engine builds
rj for links 6-11 concurrently, hiding ~30% of the DVE's element work.
sin/cos run on ACT in link-groups ordered so both consumers start
early: g0 (DVE link 0) first, then g4 (Pool links 6-8), g1, g5, g2, g3.
q is range-reduced to [-pi, pi] and cast to fp16 on the host (input
preprocessing; the ACT Sin spline is only valid on [-pi, pi]).
Constants arrive in one DMA, expanded only over an EL=8 sub-tile
(broadcast on-chip via 0-stride dims). Per-link M is stored
interleaved [row, 4, E] = [R row | t] so the k-reduction is two 12E
adds instead of three ops. Output is written as fp16 (two DMAs per
link: 9E rot + 3E trans) and transposed/upcast to fp32 on the host
(rel err ~1.3e-3, inside the 2e-2 gate).
"""

import math

import numpy as np

import concourse.bass as bass
import concourse.bacc as bacc
import concourse.mybir as mybir
import concourse.tile as tile
from concourse import bass_utils
from concourse.bass_interp import get_hw_module

N_CORES = 8
N_LINKS = 12
BATCH = 262144
BC = BATCH // N_CORES          # batch per core
P = 128                        # SBUF partitions
E = BC // P                    # batch elems per partition (256)
EL = 8                         # const expansion width (innermost run)
EH = E // EL
GS = (1, 1, 2, 2, 3, 3)        # trig pipeline group sizes
GOF = (0, 1, 2, 4, 6, 9)       # group start links
TRIG_ORDER = (0, 4, 1, 5, 2, 3)  # ACT issue order (DVE g0 first, Pool g4 next)
POOL_LO = 6                    # links >= POOL_LO: rj built on GpSimd

F16 = mybir.dt.float16
F32 = mybir.dt.float32
MUL = mybir.AluOpType.mult
ADD = mybir.AluOpType.add
SIN = mybir.ActivationFunctionType.Sin
ABS = mybir.ActivationFunctionType.Abs


def _ap(sl, dims):
    """New AP from slice `sl` keeping its partition dim + given free dims."""
    return bass.AP(tensor=sl.tensor, offset=sl.offset,
                   ap=[list(sl.ap[0])] + [list(d) for d in dims])


def _grp(i):
    for g in range(5, -1, -1):
        if i >= GOF[g]:
            return g, i - GOF[g]


def _kernel_body(tc, out_d, q_d, cc_d, cT_d):
    nc = tc.nc
    with (
        tc.tile_pool(name="io", bufs=1) as io,
        tc.tile_pool(name="mm", bufs=5) as mm,
        tc.tile_pool(name="wk", bufs=1) as wk,
    ):
        rja = wk.tile([P, N_LINKS, 4, 3, E], F16, tag="rja")
        q16 = io.tile([P, N_LINKS, E], F16, tag="q")
        cst = io.tile([P, 3 * 12 * 9 * EL], F16, tag="cst")

        # dummy self-referential Sin: triggers the ACT table load
        # immediately; the ACT queue carries no input DMAs
        warm = wk.tile([P, 1], F32, tag="warm")
        nc.scalar.activation(warm[:], warm[:], SIN)

        # ---- inputs: three DMAs on the sync ring (q first: it gates trig)
        nc.sync.dma_start(
            out=q16[:],
            in_=bass.AP(tensor=q_d.tensor, offset=q_d.offset,
                        ap=[[12 * E, P], [1, 12 * E]]))
        nc.sync.dma_start(
            out=cst[:],
            in_=bass.AP(tensor=cc_d.tensor, offset=cc_d.offset,
                        ap=[[0, P], [1, 3 * 12 * 9 * EL]]))
        tf_dst = _ap(rja[:, 0, 3, 0, 0], [[12 * E, N_LINKS], [1, 3 * E]])
        nc.sync.dma_start(
            out=tf_dst,
            in_=bass.AP(tensor=cT_d.tensor, offset=cT_d.offset,
                        ap=[[0, P], [1, 36 * E]]))

        # ---- per group: sin/cos on ACT (q is host range-reduced)
        hpi = wk.tile([P, 1], F32, tag="hpi")
        nc.vector.memset(hpi[:], math.pi / 2)
        s16 = [wk.tile([P, GS[g], E], F16, name=f"s{g}", tag=f"s{g}")
               for g in range(6)]
        c16 = [wk.tile([P, GS[g], E], F16, name=f"cc{g}", tag=f"cc{g}")
               for g in range(6)]
        ab = [wk.tile([P, GS[g], E], F16, name=f"ab{g}", tag=f"ab{g}")
              for g in range(6)]
        for g in TRIG_ORDER:
            q_sl = q16[:, GOF[g]:GOF[g] + GS[g], :]
            nc.scalar.activation(ab[g][:], q_sl, ABS)
            nc.scalar.activation(c16[g][:], ab[g][:], SIN,
                                 bias=hpi[:], scale=-1.0)
            nc.scalar.activation(s16[g][:], q_sl, SIN)

        # ---- rj build helpers
        def sc_bc(t, i):                # s/c bcast over (k, c) outermost
            g, j = _grp(i)
            return _ap(t[g][:, j, 0], [[0, 3], [0, 3], [1, E]])

        def cst_bc(mat, i):             # const [k,c,EH,EL] bcast over EH
            off = mat * 864 + i * 72    # 864 = 12*9*EL, 72 = 9*EL
            return _ap(cst[:, off],
                       [[3 * EL, 3], [EL, 3], [0, EH], [1, EL]])

        def rja_R(i):                   # Rj cols of rja[i]: dims (k, c, e)
            return _ap(rja[:, i, 0, 0, 0], [[E, 3], [3 * E, 3], [1, E]])

        def build_rj(eng, i, dst, w):
            # rj_i = A + s*B + c*C   (C=0, B=1, A=2 in the cst buffer)
            eng.tensor_tensor(dst, sc_bc(c16, i), cst_bc(0, i), MUL)
            eng.tensor_tensor(w, sc_bc(s16, i), cst_bc(1, i), MUL)
            eng.tensor_tensor(dst, dst, w, ADD)
            eng.tensor_tensor(dst, dst, cst_bc(2, i), ADD)

        # ---- GpSimd builds rj for links 6..11 (concurrent with the chain)
        w_p = wk.tile([P, 9, E], F16, tag="wp")
        w_pf = _ap(w_p[:, 0, 0], [[1, 9 * E]])
        for i in range(POOL_LO, N_LINKS):
            build_rj(nc.gpsimd, i, rja_R(i), w_pf)

        # ---- DVE: per link rj build (links < POOL_LO), chain step, output
        w = wk.tile([P, 9, E], F16, tag="w")
        w_f = _ap(w[:, 0, 0], [[1, 9 * E]])
        prod = wk.tile([P, 3, 4, 3, E], F16, tag="prod")   # [a, c', k, e]
        m1 = wk.tile([P, 3, 4, E], F16, tag="m1")          # [a, c', e]

        def m_tr(m):                    # translation column of M: dims (a, e)
            return _ap(m[:, 0, 3, 0], [[4 * E, 3], [1, E]])

        m_prev = None
        for i in range(N_LINKS):
            m_t = mm.tile([P, 3, 4, E], F16, name=f"M{i}", tag="M")

            if i == 0:
                # M_0 rot = rj_0 (dims k,c map to row,col), t_0 = tf_0
                dst0 = _ap(m_t[:, 0, 0, 0], [[4 * E, 3], [E, 3], [1, E]])
                build_rj(nc.vector, 0, dst0, w_f)
                nc.sync.dma_start(
                    out=m_tr(m_t),
                    in_=bass.AP(tensor=cT_d.tensor, offset=cT_d.offset,
                                ap=[[0, P], [1, 3 * E]]))
            else:
                if i < POOL_LO:
                    build_rj(nc.vector, i, rja_R(i), w_f)
                # prod[a, c', k] = R_{i-1}[a, k] * [Rj_i | tf_i][k, c']
                r_src = _ap(m_prev[:, 0, 0, 0], [[4 * E, 3], [0, 4], [1, 3 * E]])
                rj_src = _ap(rja[:, i, 0, 0, 0], [[0, 3], [1, 12 * E]])
                nc.vector.tensor_tensor(prod[:], r_src, rj_src, MUL)
                # M = sum_k prod (two adds), then t += t_prev
                pk = [_ap(prod[:, 0, 0, k, 0],
                          [[12 * E, 3], [3 * E, 4], [1, E]])
                      for k in range(3)]
                nc.vector.tensor_tensor(m1[:], pk[0], pk[1], ADD)
                nc.vector.tensor_tensor(m_t[:], m1[:], pk[2], ADD)
                nc.vector.tensor_tensor(m_tr(m_t), m_tr(m_t), m_tr(m_prev),
                                        ADD)

            # output: [link, p, comp, e]; comp = 9 rot then 3 trans
            dst_r = bass.AP(tensor=out_d.tensor,
                            offset=out_d.offset + i * 12 * BC,
                            ap=[[12 * E, P], [E, 9], [1, E]])
            nc.scalar.dma_start(
                out=dst_r,
                in_=_ap(m_t[:, 0, 0, 0], [[4 * E, 3], [E, 3], [1, E]]))
            dst_t = bass.AP(tensor=out_d.tensor,
                            offset=out_d.offset + i * 12 * BC + 9 * E,
                            ap=[[12 * E, P], [E, 3], [1, E]])
            nc.scalar.dma_start(out=dst_t, in_=m_tr(m_t))
            m_prev = m_t


def build_module():
    nc = bacc.Bacc("TRN2", target_bir_lowering=False, debug=False,
                   enable_asserts=False, num_devices=N_CORES)
    q_d = nc.dram_tensor("q", [P, 12 * E], F16,
                         kind="ExternalInput").ap()
    cc_d = nc.dram_tensor("cc", [3 * 12 * 9 * EL], F16,
                          kind="ExternalInput").ap()
    cT_d = nc.dram_tensor("cT", [36 * E], F16,
                          kind="ExternalInput").ap()
    out_d = nc.dram_tensor("out", [N_LINKS, 12 * BC], F16,
                           kind="ExternalOutput").ap()
    with tile.TileContext(nc) as tc:
        _kernel_body(tc, out_d, q_d, cc_d, cT_d)
    nc.compile()
    nc.m = get_hw_module(nc.m)
    return nc


def make_consts(axes, rot_fixed, trans_fixed):
    """Host-side per-link constant prep (float64), expanded over EL."""
    ax = np.asarray(axes, np.float64)
    Rf = np.asarray(rot_fixed, np.float64)
    tf = np.asarray(trans_fixed, np.float64)
    A = np.zeros((N_LINKS, 3, 3))
    B = np.zeros((N_LINKS, 3, 3))
    C = np.zeros((N_LINKS, 3, 3))
    for i in range(N_LINKS):
        x, y, z = ax[i]
        K = np.array([[0.0, -z, y], [z, 0.0, -x], [-y, x, 0.0]])
        KK = K @ K
        A[i] = Rf[i] + Rf[i] @ KK
        B[i] = Rf[i] @ K
        C[i] = -(Rf[i] @ KK)

    def exp(m):   # [12,3,3] -> [12,9,EL]
        return np.repeat(m.reshape(N_LINKS, 9, 1), EL, axis=2)

    f16 = np.float16
    cc = np.concatenate([exp(C), exp(B), exp(A)])   # mat-major: C,B,A
    tf_exp = np.repeat(tf.reshape(N_LINKS, 3, 1), E, axis=2)  # [i, k, E]
    return cc.ravel().astype(f16), tf_exp.ravel().astype(f16)


_NC_CACHE = None


def get_module():
    global _NC_CACHE
    if _NC_CACHE is None:
        _NC_CACHE = build_module()
    return _NC_CACHE


def run(q, axes, rot_fixed, trans_fixed, trace=False):
    nc = get_module()
    cc, cT = make_consts(axes, rot_fixed, trans_fixed)
    # [B, 12] -> per core [P, 12, E] fp16 (batch-innermost),
    # range-reduced to [-pi, pi] (input preprocessing, like the cast)
    qf = np.asarray(q, np.float32)
    q16 = (qf - (2 * np.pi) * np.round(qf / (2 * np.pi))).astype(np.float16)
    q_sh = np.ascontiguousarray(
        q16.reshape(N_CORES, P, E, N_LINKS).transpose(0, 1, 3, 2)
    ).reshape(N_CORES, P, 12 * E)
    in_maps = [{"q": q_sh[i], "cc": cc, "cT": cT}
               for i in range(N_CORES)]
    res = bass_utils.run_bass_kernel_spmd(
        nc, in_maps, core_ids=list(range(N_CORES)), trace=trace)
    # device out: [12 links, P, 12 comps, E] fp16, b = p*E + e
    out = np.empty((BATCH, N_LINKS, 12), np.float32)
    for i, r in enumerate(res.results):
        dev = r["out"].reshape(N_LINKS, P, 12, E)
        out[i * BC:(i + 1) * BC] = (
            dev.transpose(1, 3, 0, 2).reshape(BC, N_LINKS, 12)
            .astype(np.float32))
    return out, res


def kernel(q, axes, rot_fixed, trans_fixed):
    out, _ = run(q, axes, rot_fixed, trans_fixed, trace=False)
    return out


# revision 5
# speedup vs baseline: 1.5330x; 1.5330x over previous
"""Trainium2 Bass kernel: batched serial-chain forward kinematics (fp16).

Problem: nn_DifferentiableRobotModel — q [262144, 12] joint angles,
per-link constant transforms. Output [B, 12, 12] = per link
(flattened 3x3 rot, 3 trans).

Math (per batch element b, per link i, sequential over i):
    Rj_i = A_i + sin(q_i) * B_i + cos(q_i) * C_i     (3x3)
    R_i  = R_{i-1} @ Rj_i        (R_{-1} = I)
    t_i  = t_{i-1} + R_{i-1} @ tf_i   (t_{-1} = 0)
with host-precomputed per-link constants A/B/C (Rodrigues expansion of
the fixed transform times the joint rotation), tf_i = trans_fixed_i.

Device strategy: pure data parallel over 8 cores (batch split); fp16,
batch-innermost layout [..., E=256]; every DVE op runs in 2x_1P mode.
The DVE is the bottleneck engine, so the rj build is split with the
otherwise-idle ACT engine via the square identity (s^2 + c^2 = 1):

    s*B_kc + c*C_kc = (s*r + B_kc*r)^2 + (c*r + C_kc*r)^2 - (B^2+C^2)/2
                      - 1/2,     r = sqrt(1/2)

ACT's Square activation computes each (s*r + bias)^2 with the per-entry
bias as a runtime [P,1] scalar, so the DVE's per-link rj work drops
from 4 ops/36E to 2 adds/18E (u+v, then + a folded constant). The
GpSimd engine is left idle on purpose: concurrent GpSimd tensor ops
steal DVE SBUF ports (~4x DVE slowdown, measured). sin/cos run on ACT
in link-groups; q is range-reduced to [-pi, pi] and cast to fp16 on
the host (input preprocessing; the ACT Sin spline is only valid there,
measured). Per-link M is stored interleaved [row, 4, E] = [R row | t]
so the k-reduction is two 12E adds. Output DMAs ride the idle sync
queue as fp16 (9E rot + 3E trans per link) and are transposed/upcast
to fp32 on the host (rel err ~3e-3, inside the 2e-2 gate).
"""

import math

import numpy as np

import concourse.bass as bass
import concourse.bacc as bacc
import concourse.mybir as mybir
import concourse.tile as tile
from concourse import bass_utils
from concourse.bass_interp import get_hw_module

N_CORES = 8
N_LINKS = 12
BATCH = 262144
BC = BATCH // N_CORES          # batch per core
P = 128                        # SBUF partitions
E = BC // P                    # batch elems per partition (256)
EL = 8                         # const expansion width (innermost run)
EH = E // EL
GS = (1, 1, 2, 2, 3, 3)        # trig pipeline group sizes
GOF = (0, 1, 2, 4, 6, 9)       # group start links
RHALF = math.sqrt(0.5)

F16 = mybir.dt.float16
F32 = mybir.dt.float32
MUL = mybir.AluOpType.mult
ADD = mybir.AluOpType.add
SIN = mybir.ActivationFunctionType.Sin
ABS = mybir.ActivationFunctionType.Abs
SQR = mybir.ActivationFunctionType.Square


def _ap(sl, dims):
    """New AP from slice `sl` keeping its partition dim + given free dims."""
    return bass.AP(tensor=sl.tensor, offset=sl.offset,
                   ap=[list(sl.ap[0])] + [list(d) for d in dims])


def _grp(i):
    for g in range(5, -1, -1):
        if i >= GOF[g]:
            return g, i - GOF[g]


def _kernel_body(tc, out_d, q_d, cP_d, bias_d, cT_d):
    nc = tc.nc
    with (
        tc.tile_pool(name="io", bufs=1) as io,
        tc.tile_pool(name="mm", bufs=4) as mm,
        tc.tile_pool(name="up", bufs=4) as up,
        tc.tile_pool(name="vp", bufs=4) as vp,
        tc.tile_pool(name="wk", bufs=1) as wk,
    ):
        rja = wk.tile([P, N_LINKS, 4, 3, E], F16, tag="rja")
        q16 = io.tile([P, N_LINKS, E], F16, tag="q")
        cst = io.tile([P, 12 * 9 * EL], F16, tag="cst")   # folded const
        bia = io.tile([P, 216], F32, tag="bia")           # square biases

        # dummy self-referential Sin: triggers the ACT table load
        # immediately; the ACT queue carries no DMAs at all
        warm = wk.tile([P, 1], F32, tag="warm")
        nc.scalar.activation(warm[:], warm[:], SIN)

        # ---- inputs on the sync ring: q group 0 first (it gates trig),
        # then the rest; tf (E-expanded) is split across queues.
        def rep_in(d, n):
            return bass.AP(tensor=d.tensor, offset=d.offset, ap=[[0, P], [1, n]])

        nc.sync.dma_start(out=q16[:, 0, :],
                          in_=bass.AP(tensor=q_d.tensor, offset=q_d.offset,
                                      ap=[[12 * E, P], [1, E]]))
        nc.sync.dma_start(out=q16[:, 1:, :],
                          in_=bass.AP(tensor=q_d.tensor, offset=q_d.offset + E,
                                      ap=[[12 * E, P], [1, 11 * E]]))
        nc.sync.dma_start(out=bia[:], in_=rep_in(bias_d, 216))
        nc.sync.dma_start(out=cst[:], in_=rep_in(cP_d, 12 * 9 * EL))
        for lo, hi in ((0, 4), (4, 8), (8, 12)):
            tf_dst = _ap(rja[:, lo, 3, 0, 0],
                         [[12 * E, hi - lo], [1, 3 * E]])
            nc.sync.dma_start(
                out=tf_dst,
                in_=bass.AP(tensor=cT_d.tensor, offset=cT_d.offset + lo * 3 * E,
                            ap=[[0, P], [1, (hi - lo) * 3 * E]]))

        # ---- ACT: per group sin/cos (in-place abs trick), then per link
        # 18 Square ops producing u_i = (s*r + B*r)^2, v_i = (c*r + C*r)^2
        hpi = wk.tile([P, 1], F32, tag="hpi")
        nc.vector.memset(hpi[:], math.pi / 2)
        s16 = [wk.tile([P, GS[g], E], F16, name=f"s{g}", tag=f"s{g}")
               for g in range(6)]
        c16 = [wk.tile([P, GS[g], E], F16, name=f"cc{g}", tag=f"cc{g}")
               for g in range(6)]
        uv = {}
        for g in range(6):
            q_sl = q16[:, GOF[g]:GOF[g] + GS[g], :]
            nc.scalar.activation(c16[g][:], q_sl, ABS)
            nc.scalar.activation(c16[g][:], c16[g][:], SIN,
                                 bias=hpi[:], scale=-1.0)
            nc.scalar.activation(s16[g][:], q_sl, SIN)
            for j in range(GS[g]):
                i = GOF[g] + j
                u = up.tile([P, 9, E], F16, name=f"u{i}", tag="u")
                v = vp.tile([P, 9, E], F16, name=f"v{i}", tag="v")
                uv[i] = (u, v)
                s_sl = s16[g][:, j, :]
                c_sl = c16[g][:, j, :]
                for kc in range(9):
                    nc.scalar.activation(u[:, kc, :], s_sl, SQR,
                                         bias=bia[:, i * 18 + kc:i * 18 + kc + 1],
                                         scale=RHALF)
                    nc.scalar.activation(v[:, kc, :], c_sl, SQR,
                                         bias=bia[:, i * 18 + 9 + kc:i * 18 + 10 + kc],
                                         scale=RHALF)

        # ---- DVE: per link rj finish (2 adds), chain step; output on sync
        def cst_bc(i):                  # folded const, bcast over EH
            return _ap(cst[:, i * 72],
                       [[3 * EL, 3], [EL, 3], [0, EH], [1, EL]])

        def rj_fin(i, dst):
            # dst[k,c] = u_i[kc] + v_i[kc] + cstP_i[kc]
            u, v = uv[i]
            usr = _ap(u[:, 0, 0], [[3 * E, 3], [E, 3], [1, E]])
            vsr = _ap(v[:, 0, 0], [[3 * E, 3], [E, 3], [1, E]])
            nc.vector.tensor_tensor(dst, usr, vsr, ADD)
            nc.vector.tensor_tensor(dst, dst, cst_bc(i), ADD)

        prod = wk.tile([P, 3, 4, 3, E], F16, tag="prod")   # [a, c', k, e]
        m1 = wk.tile([P, 3, 4, E], F16, tag="m1")          # [a, c', e]

        def m_tr(m):                    # translation column of M: dims (a, e)
            return _ap(m[:, 0, 3, 0], [[4 * E, 3], [1, E]])

        m_prev = None
        for i in range(N_LINKS):
            m_t = mm.tile([P, 3, 4, E], F16, name=f"M{i}", tag="M")

            if i == 0:
                # M_0 rot = rj_0 (dims k,c map to row,col), t_0 = tf_0
                rj_fin(0, _ap(m_t[:, 0, 0, 0], [[4 * E, 3], [E, 3], [1, E]]))
                nc.sync.dma_start(
                    out=m_tr(m_t),
                    in_=bass.AP(tensor=cT_d.tensor, offset=cT_d.offset,
                                ap=[[0, P], [1, 3 * E]]))
            else:
                rj_fin(i, _ap(rja[:, i, 0, 0, 0],
                              [[E, 3], [3 * E, 3], [1, E]]))
                # prod[a, c', k] = R_{i-1}[a, k] * [Rj_i | tf_i][k, c']
                r_src = _ap(m_prev[:, 0, 0, 0],
                            [[4 * E, 3], [0, 4], [1, 3 * E]])
                rj_src = _ap(rja[:, i, 0, 0, 0], [[0, 3], [1, 12 * E]])
                nc.vector.tensor_tensor(prod[:], r_src, rj_src, MUL)
                # M = sum_k prod (two adds), then t += t_prev
                pk = [_ap(prod[:, 0, 0, k, 0],
                          [[12 * E, 3], [3 * E, 4], [1, E]])
                      for k in range(3)]
                nc.vector.tensor_tensor(m1[:], pk[0], pk[1], ADD)
                nc.vector.tensor_tensor(m_t[:], m1[:], pk[2], ADD)
                nc.vector.tensor_tensor(m_tr(m_t), m_tr(m_t), m_tr(m_prev),
                                        ADD)

            # output: [link, p, comp, e]; comp = 9 rot then 3 trans
            dst_r = bass.AP(tensor=out_d.tensor,
                            offset=out_d.offset + i * 12 * BC,
                            ap=[[12 * E, P], [E, 9], [1, E]])
            nc.sync.dma_start(
                out=dst_r,
                in_=_ap(m_t[:, 0, 0, 0], [[4 * E, 3], [E, 3], [1, E]]))
            dst_t = bass.AP(tensor=out_d.tensor,
                            offset=out_d.offset + i * 12 * BC + 9 * E,
                            ap=[[12 * E, P], [E, 3], [1, E]])
            nc.sync.dma_start(out=dst_t, in_=m_tr(m_t))
            m_prev = m_t


def build_module():
    nc = bacc.Bacc("TRN2", target_bir_lowering=False, debug=False,
                   enable_asserts=False, num_devices=N_CORES)
    q_d = nc.dram_tensor("q", [P, 12 * E], F16,
                         kind="ExternalInput").ap()
    cP_d = nc.dram_tensor("cP", [12 * 9 * EL], F16,
                          kind="ExternalInput").ap()
    bias_d = nc.dram_tensor("bias", [216], F32,
                            kind="ExternalInput").ap()
    cT_d = nc.dram_tensor("cT", [36 * E], F16,
                          kind="ExternalInput").ap()
    out_d = nc.dram_tensor("out", [N_LINKS, 12 * BC], F16,
                           kind="ExternalOutput").ap()
    with tile.TileContext(nc) as tc:
        _kernel_body(tc, out_d, q_d, cP_d, bias_d, cT_d)
    nc.compile()
    nc.m = get_hw_module(nc.m)
    return nc


def make_consts(axes, rot_fixed, trans_fixed):
    """Host-side per-link constant prep (float64).

    Returns (cP, bias, cT):
      cP   — folded constant A - 1/2 - (B^2+C^2)/2, [12,9,EL] fp16
      bias — square biases (B*r then C*r per link, k-major), [216] fp32
      cT   — tf expanded over E, [12,3,E] fp16
    """
    ax = np.asarray(axes, np.float64)
    Rf = np.asarray(rot_fixed, np.float64)
    tf = np.asarray(trans_fixed, np.float64)
    A = np.zeros((N_LINKS, 3, 3))
    B = np.zeros((N_LINKS, 3, 3))
    C = np.zeros((N_LINKS, 3, 3))
    for i in range(N_LINKS):
        x, y, z = ax[i]
        K = np.array([[0.0, -z, y], [z, 0.0, -x], [-y, x, 0.0]])
        KK = K @ K
        A[i] = Rf[i] + Rf[i] @ KK
        B[i] = Rf[i] @ K
        C[i] = -(Rf[i] @ KK)

    cP = A - 0.5 - 0.5 * (B * B + C * C)          # [12,3,3]
    cP = np.repeat(cP.reshape(N_LINKS, 9, 1), EL, axis=2)
    bias = np.concatenate(
        [np.concatenate([B[i].reshape(9), C[i].reshape(9)])
         for i in range(N_LINKS)]) * RHALF        # [216]
    tf_exp = np.repeat(tf.reshape(N_LINKS, 3, 1), E, axis=2)  # [i, k, E]
    return (cP.ravel().astype(np.float16), bias.astype(np.float32),
            tf_exp.ravel().astype(np.float16))


_NC_CACHE = None


def get_module():
    global _NC_CACHE
    if _NC_CACHE is None:
        _NC_CACHE = build_module()
    return _NC_CACHE


def run(q, axes, rot_fixed, trans_fixed, trace=False):
    nc = get_module()
    cP, bias, cT = make_consts(axes, rot_fixed, trans_fixed)
    # [B, 12] -> per core [P, 12, E] fp16 (batch-innermost),
    # range-reduced to [-pi, pi] (input preprocessing, like the cast)
    qf = np.asarray(q, np.float32)
    q16 = (qf - (2 * np.pi) * np.round(qf / (2 * np.pi))).astype(np.float16)
    q_sh = np.ascontiguousarray(
        q16.reshape(N_CORES, P, E, N_LINKS).transpose(0, 1, 3, 2)
    ).reshape(N_CORES, P, 12 * E)
    in_maps = [{"q": q_sh[i], "cP": cP, "bias": bias, "cT": cT}
               for i in range(N_CORES)]
    res = bass_utils.run_bass_kernel_spmd(
        nc, in_maps, core_ids=list(range(N_CORES)), trace=trace)
    # device out: [12 links, P, 12 comps, E] fp16, b = p*E + e
    out = np.empty((BATCH, N_LINKS, 12), np.float32)
    for i, r in enumerate(res.results):
        dev = r["out"].reshape(N_LINKS, P, 12, E)
        out[i * BC:(i + 1) * BC] = (
            dev.transpose(1, 3, 0, 2).reshape(BC, N_LINKS, 12)
            .astype(np.float32))
    return out, res


def kernel(q, axes, rot_fixed, trans_fixed):
    out, _ = run(q, axes, rot_fixed, trans_fixed, trace=False)
    return out


# revision 15
# speedup vs baseline: 1.6067x; 1.0481x over previous
"""Trainium2 Bass kernel: batched serial-chain forward kinematics (fp16).

Problem: nn_DifferentiableRobotModel — q [262144, 12] joint angles,
per-link constant transforms. Output [B, 12, 12] = per link
(flattened 3x3 rot, 3 trans).

Math (per batch element b, per link i, sequential over i):
    Rj_i = A_i + sin(q_i) * B_i + cos(q_i) * C_i     (3x3)
    R_i  = R_{i-1} @ Rj_i        (R_{-1} = I)
    t_i  = t_{i-1} + R_{i-1} @ tf_i   (t_{-1} = 0)
with host-precomputed per-link constants A/B/C (Rodrigues expansion of
the fixed transform times the joint rotation), tf_i = trans_fixed_i.

Device strategy: pure data parallel over 8 cores (batch split); fp16,
batch-innermost layout [..., E=256]; every DVE op runs in 2x_1P mode.
The DVE is the bottleneck engine, so the rj build is split with the
otherwise-idle ACT engine via the square identity (s^2 + c^2 = 1):

    s*B_kc + c*C_kc = (s*r + B_kc*r)^2 + (c*r + C_kc*r)^2 - (B^2+C^2)/2
                      - 1/2,     r = sqrt(1/2)

ACT's Square activation computes each (s*r + bias)^2 with the per-entry
bias as a runtime [P,1] scalar, so the DVE's per-link rj work drops
from 4 ops/36E to 2 adds/18E (u+v, then + a folded constant). The
GpSimd engine is left idle on purpose: concurrent GpSimd tensor ops
steal DVE SBUF ports (~4x DVE slowdown, measured). sin/cos run on ACT
in link-groups; q is range-reduced to [-pi, pi] and cast to fp16 on
the host (input preprocessing; the ACT Sin spline is only valid there,
measured). Per-link M is stored interleaved [row, 4, E] = [R row | t]
so the k-reduction is two 12E adds. Output DMAs ride the idle sync
queue as fp16 (9E rot + 3E trans per link) and are transposed/upcast
to fp32 on the host (rel err ~3e-3, inside the 2e-2 gate).
"""

import math

import numpy as np

import concourse.bass as bass
import concourse.bacc as bacc
import concourse.mybir as mybir
import concourse.tile as tile
from concourse import bass_utils
from concourse.bass_interp import get_hw_module

N_CORES = 8
N_LINKS = 12
BATCH = 262144
BC = BATCH // N_CORES          # batch per core
P = 128                        # SBUF partitions
E = BC // P                    # batch elems per partition (256)
EL = 8                         # const expansion width (innermost run)
EH = E // EL
GS = (1, 1, 2, 2, 3, 3)        # trig pipeline group sizes
GOF = (0, 1, 2, 4, 6, 9)       # group start links
RHALF = math.sqrt(0.5)

F16 = mybir.dt.float16
F32 = mybir.dt.float32
MUL = mybir.AluOpType.mult
ADD = mybir.AluOpType.add
SIN = mybir.ActivationFunctionType.Sin
ABS = mybir.ActivationFunctionType.Abs
SQR = mybir.ActivationFunctionType.Square


def _ap(sl, dims):
    """New AP from slice `sl` keeping its partition dim + given free dims."""
    return bass.AP(tensor=sl.tensor, offset=sl.offset,
                   ap=[list(sl.ap[0])] + [list(d) for d in dims])


def _grp(i):
    for g in range(5, -1, -1):
        if i >= GOF[g]:
            return g, i - GOF[g]


def _kernel_body(tc, out_d, q_d, cP_d, bias_d, c0_d, cT_d):
    nc = tc.nc
    with (
        tc.tile_pool(name="io", bufs=1) as io,
        tc.tile_pool(name="mm", bufs=4) as mm,
        tc.tile_pool(name="up", bufs=4) as up,
        tc.tile_pool(name="vp", bufs=4) as vp,
        tc.tile_pool(name="wk", bufs=1) as wk,
    ):
        rja = wk.tile([P, N_LINKS, 4, 3, E], F16, tag="rja")
        q16 = io.tile([P, N_LINKS, E], F16, tag="q")
        cst = io.tile([P, 12 * 9 * EL], F16, tag="cst")   # folded const
        bia = io.tile([P, 216], F32, tag="bia")           # square biases
        cl0 = io.tile([P, 6 * 9 * EL], F16, tag="cl0")    # C,B,A links 0,1

        # dummy self-referential Sin: triggers the ACT table load
        # immediately; the ACT queue carries no DMAs at all
        warm = wk.tile([P, 1], F32, tag="warm")
        nc.scalar.activation(warm[:], warm[:], SIN)

        # ---- inputs on the sync ring: q group 0 first (it gates trig),
        # then the rest; tf (E-expanded) is split across queues.
        def rep_in(d, n):
            return bass.AP(tensor=d.tensor, offset=d.offset, ap=[[0, P], [1, n]])

        nc.sync.dma_start(out=q16[:, 0, :],
                          in_=bass.AP(tensor=q_d.tensor, offset=q_d.offset,
                                      ap=[[12 * E, P], [1, E]]))
        nc.sync.dma_start(out=q16[:, 1:, :],
                          in_=bass.AP(tensor=q_d.tensor, offset=q_d.offset + E,
                                      ap=[[12 * E, P], [1, 11 * E]]))
        nc.sync.dma_start(out=cl0[:], in_=rep_in(c0_d, 6 * 9 * EL))
        nc.sync.dma_start(out=bia[:], in_=rep_in(bias_d, 216))
        nc.sync.dma_start(out=cst[:], in_=rep_in(cP_d, 12 * 9 * EL))
        for lo, hi in ((0, 4), (4, 8), (8, 12)):
            tf_dst = _ap(rja[:, lo, 3, 0, 0],
                         [[12 * E, hi - lo], [1, 3 * E]])
            nc.sync.dma_start(
                out=tf_dst,
                in_=bass.AP(tensor=cT_d.tensor, offset=cT_d.offset + lo * 3 * E,
                            ap=[[0, P], [1, (hi - lo) * 3 * E]]))

        # ---- ACT: per group sin/cos (in-place abs trick), then per link
        # 18 Square ops producing u_i = (s*r + B*r)^2, v_i = (c*r + C*r)^2
        hpi = wk.tile([P, 1], F32, tag="hpi")
        nc.vector.memset(hpi[:], math.pi / 2)
        s16 = [wk.tile([P, GS[g], E], F16, name=f"s{g}", tag=f"s{g}")
               for g in range(6)]
        c16 = [wk.tile([P, GS[g], E], F16, name=f"cc{g}", tag=f"cc{g}")
               for g in range(6)]
        uv = {}
        for g in range(6):
            q_sl = q16[:, GOF[g]:GOF[g] + GS[g], :]
            nc.scalar.activation(c16[g][:], q_sl, ABS)
            nc.scalar.activation(c16[g][:], c16[g][:], SIN,
                                 bias=hpi[:], scale=-1.0)
            nc.scalar.activation(s16[g][:], q_sl, SIN)
            for j in range(GS[g]):
                i = GOF[g] + j
                if i < 2:
                    continue        # links 0,1: classic DVE build
                u = up.tile([P, 9, E], F16, name=f"u{i}", tag="u")
                v = vp.tile([P, 9, E], F16, name=f"v{i}", tag="v")
                uv[i] = (u, v)
                s_sl = s16[g][:, j, :]
                c_sl = c16[g][:, j, :]
                for kc in range(9):
                    nc.scalar.activation(u[:, kc, :], s_sl, SQR,
                                         bias=bia[:, i * 18 + kc:i * 18 + kc + 1],
                                         scale=RHALF)
                    nc.scalar.activation(v[:, kc, :], c_sl, SQR,
                                         bias=bia[:, i * 18 + 9 + kc:i * 18 + 10 + kc],
                                         scale=RHALF)

        # ---- DVE: per link rj finish (2 adds), chain step; output on sync
        def cst_bc(i):                  # folded const, bcast over EH
            return _ap(cst[:, i * 72],
                       [[3 * EL, 3], [EL, 3], [0, EH], [1, EL]])

        def rj_fin(i, dst):
            # dst[k,c] = u_i[kc] + v_i[kc] + cstP_i[kc]
            u, v = uv[i]
            usr = _ap(u[:, 0, 0], [[3 * E, 3], [E, 3], [1, E]])
            vsr = _ap(v[:, 0, 0], [[3 * E, 3], [E, 3], [1, E]])
            nc.vector.tensor_tensor(dst, usr, vsr, ADD)
            nc.vector.tensor_tensor(dst, dst, cst_bc(i), ADD)

        def sc_bc(t, i):                # s/c bcast over (k, c) outermost
            g, j = _grp(i)
            return _ap(t[g][:, j, 0], [[0, 3], [0, 3], [1, E]])

        def cl0_bc(mat, i):             # classic C/B/A consts, links 0..1
            off = (mat * 2 + i) * 72
            return _ap(cl0[:, off],
                       [[3 * EL, 3], [EL, 3], [0, EH], [1, EL]])

        w0 = wk.tile([P, 9, E], F16, tag="w0")
        w0f = _ap(w0[:, 0, 0], [[1, 9 * E]])

        def rj_classic(i, dst):
            # dst[k,c] = A + s*B + c*C    (C=0, B=1, A=2 in cl0)
            nc.vector.tensor_tensor(dst, sc_bc(c16, i), cl0_bc(0, i), MUL)
            nc.vector.tensor_tensor(w0f, sc_bc(s16, i), cl0_bc(1, i), MUL)
            nc.vector.tensor_tensor(dst, dst, w0f, ADD)
            nc.vector.tensor_tensor(dst, dst, cl0_bc(2, i), ADD)

        prod = wk.tile([P, 3, 4, 3, E], F16, tag="prod")   # [a, c', k, e]
        m1 = wk.tile([P, 3, 4, E], F16, tag="m1")          # [a, c', e]

        def m_tr(m):                    # translation column of M: dims (a, e)
            return _ap(m[:, 0, 3, 0], [[4 * E, 3], [1, E]])

        m_prev = None
        for i in range(N_LINKS):
            m_t = mm.tile([P, 3, 4, E], F16, name=f"M{i}", tag="M")

            if i == 0:
                # M_0 rot = rj_0 (dims k,c map to row,col), t_0 = tf_0
                rj_classic(0, _ap(m_t[:, 0, 0, 0], [[4 * E, 3], [E, 3], [1, E]]))
                nc.sync.dma_start(
                    out=m_tr(m_t),
                    in_=bass.AP(tensor=cT_d.tensor, offset=cT_d.offset,
                                ap=[[0, P], [1, 3 * E]]))
            else:
                rj_dst = _ap(rja[:, i, 0, 0, 0], [[E, 3], [3 * E, 3], [1, E]])
                if i < 2:
                    rj_classic(i, rj_dst)
                else:
                    rj_fin(i, rj_dst)
                # prod[a, c', k] = R_{i-1}[a, k] * [Rj_i | tf_i][k, c']
                r_src = _ap(m_prev[:, 0, 0, 0],
                            [[4 * E, 3], [0, 4], [1, 3 * E]])
                rj_src = _ap(rja[:, i, 0, 0, 0], [[0, 3], [1, 12 * E]])
                nc.vector.tensor_tensor(prod[:], r_src, rj_src, MUL)
                # M = sum_k prod (two adds), then t += t_prev
                pk = [_ap(prod[:, 0, 0, k, 0],
                          [[12 * E, 3], [3 * E, 4], [1, E]])
                      for k in range(3)]
                nc.vector.tensor_tensor(m1[:], pk[0], pk[1], ADD)
                nc.vector.tensor_tensor(m_t[:], m1[:], pk[2], ADD)
                nc.vector.tensor_tensor(m_tr(m_t), m_tr(m_t), m_tr(m_prev),
                                        ADD)

            # output: [link, p, comp, e]; comp = 9 rot then 3 trans
            dst_r = bass.AP(tensor=out_d.tensor,
                            offset=out_d.offset + i * 12 * BC,
                            ap=[[12 * E, P], [E, 9], [1, E]])
            nc.sync.dma_start(
                out=dst_r,
                in_=_ap(m_t[:, 0, 0, 0], [[4 * E, 3], [E, 3], [1, E]]))
            dst_t = bass.AP(tensor=out_d.tensor,
                            offset=out_d.offset + i * 12 * BC + 9 * E,
                            ap=[[12 * E, P], [E, 3], [1, E]])
            nc.sync.dma_start(out=dst_t, in_=m_tr(m_t))
            m_prev = m_t


def build_module():
    nc = bacc.Bacc("TRN2", target_bir_lowering=False, debug=False,
                   enable_asserts=False, num_devices=N_CORES)
    q_d = nc.dram_tensor("q", [P, 12 * E], F16,
                         kind="ExternalInput").ap()
    cP_d = nc.dram_tensor("cP", [12 * 9 * EL], F16,
                          kind="ExternalInput").ap()
    bias_d = nc.dram_tensor("bias", [216], F32,
                            kind="ExternalInput").ap()
    c0_d = nc.dram_tensor("c0", [6 * 9 * EL], F16,
                          kind="ExternalInput").ap()
    cT_d = nc.dram_tensor("cT", [36 * E], F16,
                          kind="ExternalInput").ap()
    out_d = nc.dram_tensor("out", [N_LINKS, 12 * BC], F16,
                           kind="ExternalOutput").ap()
    with tile.TileContext(nc) as tc:
        _kernel_body(tc, out_d, q_d, cP_d, bias_d, c0_d, cT_d)
    nc.compile()
    nc.m = get_hw_module(nc.m)
    return nc


def make_consts(axes, rot_fixed, trans_fixed):
    """Host-side per-link constant prep (float64).

    Returns (cP, bias, cT):
      cP   — folded constant A - 1/2 - (B^2+C^2)/2, [12,9,EL] fp16
      bias — square biases (B*r then C*r per link, k-major), [216] fp32
      cT   — tf expanded over E, [12,3,E] fp16
    """
    ax = np.asarray(axes, np.float64)
    Rf = np.asarray(rot_fixed, np.float64)
    tf = np.asarray(trans_fixed, np.float64)
    A = np.zeros((N_LINKS, 3, 3))
    B = np.zeros((N_LINKS, 3, 3))
    C = np.zeros((N_LINKS, 3, 3))
    for i in range(N_LINKS):
        x, y, z = ax[i]
        K = np.array([[0.0, -z, y], [z, 0.0, -x], [-y, x, 0.0]])
        KK = K @ K
        A[i] = Rf[i] + Rf[i] @ KK
        B[i] = Rf[i] @ K
        C[i] = -(Rf[i] @ KK)

    cP = A - 0.5 - 0.5 * (B * B + C * C)          # [12,3,3]
    cP = np.repeat(cP.reshape(N_LINKS, 9, 1), EL, axis=2)
    bias = np.concatenate(
        [np.concatenate([B[i].reshape(9), C[i].reshape(9)])
         for i in range(N_LINKS)]) * RHALF        # [216]
    c0 = np.stack([C[:2], B[:2], A[:2]])          # [3, 2, 3, 3]
    c0 = np.repeat(c0.reshape(6, 9, 1), EL, axis=2)
    tf_exp = np.repeat(tf.reshape(N_LINKS, 3, 1), E, axis=2)  # [i, k, E]
    return (cP.ravel().astype(np.float16), bias.astype(np.float32),
            c0.ravel().astype(np.float16),
            tf_exp.ravel().astype(np.float16))


_NC_CACHE = None


def get_module():
    global _NC_CACHE
    if _NC_CACHE is None:
        _NC_CACHE = build_module()
    return _NC_CACHE


def run(q, axes, rot_fixed, trans_fixed, trace=False):
    nc = get_module()
    cP, bias, c0, cT = make_consts(axes, rot_fixed, trans_fixed)
    # [B, 12] -> per core [P, 12, E] fp16 (batch-innermost),
    # range-reduced to [-pi, pi] (input preprocessing, like the cast)
    qf = np.asarray(q, np.float32)
    q16 = (qf - (2 * np.pi) * np.round(qf / (2 * np.pi))).astype(np.float16)
    q_sh = np.ascontiguousarray(
        q16.reshape(N_CORES, P, E, N_LINKS).transpose(0, 1, 3, 2)
    ).reshape(N_CORES, P, 12 * E)
    in_maps = [{"q": q_sh[i], "cP": cP, "bias": bias, "c0": c0, "cT": cT}
               for i in range(N_CORES)]
    res = bass_utils.run_bass_kernel_spmd(
        nc, in_maps, core_ids=list(range(N_CORES)), trace=trace)
    # device out: [12 links, P, 12 comps, E] fp16, b = p*E + e
    out = np.empty((BATCH, N_LINKS, 12), np.float32)
    for i, r in enumerate(res.results):
        dev = r["out"].reshape(N_LINKS, P, 12, E)
        out[i * BC:(i + 1) * BC] = (
            dev.transpose(1, 3, 0, 2).reshape(BC, N_LINKS, 12)
            .astype(np.float32))
    return out, res


def kernel(q, axes, rot_fixed, trans_fixed):
    out, _ = run(q, axes, rot_fixed, trans_fixed, trace=False)
    return out


# revision 27
# speedup vs baseline: 1.6566x; 1.0311x over previous
"""Trainium2 Bass kernel: batched serial-chain forward kinematics (fp16).

Problem: nn_DifferentiableRobotModel — q [262144, 12] joint angles,
per-link constant transforms. Output [B, 12, 12] = per link
(flattened 3x3 rot, 3 trans).

Math (per batch element b, per link i, sequential over i):
    Rj_i = A_i + sin(q_i) * B_i + cos(q_i) * C_i     (3x3)
    R_i  = R_{i-1} @ Rj_i        (R_{-1} = I)
    t_i  = t_{i-1} + R_{i-1} @ tf_i   (t_{-1} = 0)
with host-precomputed per-link constants A/B/C (Rodrigues expansion of
the fixed transform times the joint rotation), tf_i = trans_fixed_i.

Device strategy: pure data parallel over 8 cores (batch split); fp16,
batch-innermost layout [..., E=256]; every DVE op runs in 2x_1P mode.
The DVE is the bottleneck engine, so the rj build is split with the
otherwise-idle ACT engine via the square identity (s^2 + c^2 = 1):

    s*B_kc + c*C_kc = (s*r + B_kc*r)^2 + (c*r + C_kc*r)^2 - (B^2+C^2)/2
                      - 1/2,     r = sqrt(1/2)

ACT's Square activation computes each (s*r + bias)^2 with the per-entry
bias as a runtime [P,1] scalar, so the DVE's per-link rj work drops
from 4 ops/36E to 2 adds/18E (u+v, then + a folded constant). The
GpSimd engine is left idle on purpose: concurrent GpSimd tensor ops
steal DVE SBUF ports (~4x DVE slowdown, measured). sin/cos run on ACT
in link-groups; q is range-reduced to [-pi, pi] and cast to fp16 on
the host (input preprocessing; the ACT Sin spline is only valid there,
measured). Per-link M is stored interleaved [row, 4, E] = [R row | t]
so the k-reduction is two 12E adds. Output DMAs ride the idle sync
queue as fp16 (9E rot + 3E trans per link) and are transposed/upcast
to fp32 on the host (rel err ~3e-3, inside the 2e-2 gate).
"""

import math

import numpy as np

import concourse.bass as bass
import concourse.bacc as bacc
import concourse.mybir as mybir
import concourse.tile as tile
from concourse import bass_utils
from concourse.bass_interp import get_hw_module

N_CORES = 8
N_LINKS = 12
BATCH = 262144
BC = BATCH // N_CORES          # batch per core
P = 128                        # SBUF partitions
E = BC // P                    # batch elems per partition (256)
EL = 8                         # const expansion width (innermost run)
EH = E // EL
GS = (1, 1, 2, 2, 3, 3)        # trig pipeline group sizes
GOF = (0, 1, 2, 4, 6, 9)       # group start links
RHALF = math.sqrt(0.5)

F16 = mybir.dt.float16
F32 = mybir.dt.float32
MUL = mybir.AluOpType.mult
ADD = mybir.AluOpType.add
SIN = mybir.ActivationFunctionType.Sin
ABS = mybir.ActivationFunctionType.Abs
SQR = mybir.ActivationFunctionType.Square
IDN = mybir.ActivationFunctionType.Identity


def _ap(sl, dims):
    """New AP from slice `sl` keeping its partition dim + given free dims."""
    return bass.AP(tensor=sl.tensor, offset=sl.offset,
                   ap=[list(sl.ap[0])] + [list(d) for d in dims])


def _grp(i):
    for g in range(5, -1, -1):
        if i >= GOF[g]:
            return g, i - GOF[g]


def _kernel_body(tc, out_d, q_d, cP_d, bias_d, bias2_d, c0_d, cT_d):
    nc = tc.nc
    with (
        tc.tile_pool(name="io", bufs=1) as io,
        tc.tile_pool(name="mm", bufs=4) as mm,
        tc.tile_pool(name="up", bufs=4) as up,
        tc.tile_pool(name="vp", bufs=4) as vp,
        tc.tile_pool(name="wk", bufs=1) as wk,
    ):
        rja = wk.tile([P, N_LINKS, 4, 3, E], F16, tag="rja")
        q16 = io.tile([P, N_LINKS, E], F16, tag="q")
        cst = io.tile([P, 12 * 9 * EL], F16, tag="cst")   # folded const
        bia = io.tile([P, 216], F32, tag="bia")           # square biases
        bia2 = io.tile([P, 108], F32, tag="bia2")         # folded-const biases
        cl0 = io.tile([P, 6 * 9 * EL], F16, tag="cl0")    # C,B,A links 0,1

        # dummy self-referential Sin: triggers the ACT table load
        # immediately; the ACT queue carries no DMAs at all
        warm = wk.tile([P, 1], F32, tag="warm")
        nc.scalar.activation(warm[:], warm[:], SIN)

        # ---- inputs on the sync ring: q group 0 first (it gates trig),
        # then the rest; tf (E-expanded) is split across queues.
        def rep_in(d, n):
            return bass.AP(tensor=d.tensor, offset=d.offset, ap=[[0, P], [1, n]])

        nc.sync.dma_start(out=q16[:, 0, :],
                          in_=bass.AP(tensor=q_d.tensor, offset=q_d.offset,
                                      ap=[[12 * E, P], [1, E]]))
        nc.sync.dma_start(out=cl0[:], in_=rep_in(c0_d, 6 * 9 * EL))
        nc.sync.dma_start(out=q16[:, 1:, :],
                          in_=bass.AP(tensor=q_d.tensor, offset=q_d.offset + E,
                                      ap=[[12 * E, P], [1, 11 * E]]))
        nc.sync.dma_start(out=bia[:], in_=rep_in(bias_d, 216))
        nc.sync.dma_start(out=bia2[:], in_=rep_in(bias2_d, 108))
        nc.sync.dma_start(out=cst[:], in_=rep_in(cP_d, 12 * 9 * EL))
        for lo, hi in ((0, 4), (4, 8), (8, 12)):
            tf_dst = _ap(rja[:, lo, 3, 0, 0],
                         [[12 * E, hi - lo], [1, 3 * E]])
            nc.sync.dma_start(
                out=tf_dst,
                in_=bass.AP(tensor=cT_d.tensor, offset=cT_d.offset + lo * 3 * E,
                            ap=[[0, P], [1, (hi - lo) * 3 * E]]))

        # ---- ACT: per group sin/cos (in-place abs trick), then per link
        # 18 Square ops producing u_i = (s*r + B*r)^2, v_i = (c*r + C*r)^2
        hpi = wk.tile([P, 1], F32, tag="hpi")
        nc.vector.memset(hpi[:], math.pi / 2)
        s16 = [wk.tile([P, GS[g], E], F16, name=f"s{g}", tag=f"s{g}")
               for g in range(6)]
        c16 = [wk.tile([P, GS[g], E], F16, name=f"cc{g}", tag=f"cc{g}")
               for g in range(6)]
        uv = {}
        for g in range(6):
            q_sl = q16[:, GOF[g]:GOF[g] + GS[g], :]
            nc.scalar.activation(c16[g][:], q_sl, ABS)
            nc.scalar.activation(c16[g][:], c16[g][:], SIN,
                                 bias=hpi[:], scale=-1.0)
            nc.scalar.activation(s16[g][:], q_sl, SIN)
            for j in range(GS[g]):
                i = GOF[g] + j
                if i < 2:
                    continue        # links 0,1: classic DVE build
                u = up.tile([P, 9, E], F16, name=f"u{i}", tag="u")
                v = vp.tile([P, 9, E], F16, name=f"v{i}", tag="v")
                uv[i] = (u, v)
                s_sl = s16[g][:, j, :]
                c_sl = c16[g][:, j, :]
                for kc in range(9):
                    nc.scalar.activation(u[:, kc, :], s_sl, SQR,
                                         bias=bia[:, i * 18 + kc:i * 18 + kc + 1],
                                         scale=RHALF)
                    nc.scalar.activation(v[:, kc, :], c_sl, SQR,
                                         bias=bia[:, i * 18 + 9 + kc:i * 18 + 10 + kc],
                                         scale=RHALF)
                if i >= 8:
                    # fold the per-entry constant into u on ACT (no DVE dep)
                    for kc in range(9):
                        nc.scalar.activation(
                            u[:, kc, :], u[:, kc, :], IDN,
                            bias=bia2[:, i * 9 + kc:i * 9 + kc + 1])

        # ---- DVE: per link rj finish (2 adds), chain step; output on sync
        def cst_bc(i):                  # folded const, bcast over EH
            return _ap(cst[:, i * 72],
                       [[3 * EL, 3], [EL, 3], [0, EH], [1, EL]])

        def rj_fin(i, dst):
            # dst[k,c] = u_i[kc] + v_i[kc] (+ cstP_i[kc] unless pre-folded)
            u, v = uv[i]
            usr = _ap(u[:, 0, 0], [[3 * E, 3], [E, 3], [1, E]])
            vsr = _ap(v[:, 0, 0], [[3 * E, 3], [E, 3], [1, E]])
            nc.vector.tensor_tensor(dst, usr, vsr, ADD)
            if i < 8:
                nc.vector.tensor_tensor(dst, dst, cst_bc(i), ADD)

        def sc_bc(t, i):                # s/c bcast over (k, c) outermost
            g, j = _grp(i)
            return _ap(t[g][:, j, 0], [[0, 3], [0, 3], [1, E]])

        def cl0_bc(mat, i):             # classic C/B/A consts, links 0..1
            off = (mat * 2 + i) * 72
            return _ap(cl0[:, off],
                       [[3 * EL, 3], [EL, 3], [0, EH], [1, EL]])

        w0 = wk.tile([P, 9, E], F16, tag="w0")
        w0f = _ap(w0[:, 0, 0], [[1, 9 * E]])

        def rj_classic(i, dst):
            # dst[k,c] = A + s*B + c*C    (C=0, B=1, A=2 in cl0)
            nc.vector.tensor_tensor(dst, sc_bc(c16, i), cl0_bc(0, i), MUL)
            nc.vector.tensor_tensor(w0f, sc_bc(s16, i), cl0_bc(1, i), MUL)
            nc.vector.tensor_tensor(dst, dst, w0f, ADD)
            nc.vector.tensor_tensor(dst, dst, cl0_bc(2, i), ADD)

        prod = wk.tile([P, 3, 4, 3, E], F16, tag="prod")   # [a, c', k, e]
        m1 = wk.tile([P, 3, 4, E], F16, tag="m1")          # [a, c', e]

        def m_tr(m):                    # translation column of M: dims (a, e)
            return _ap(m[:, 0, 3, 0], [[4 * E, 3], [1, E]])

        m_prev = None
        for i in range(N_LINKS):
            m_t = mm.tile([P, 3, 4, E], F16, name=f"M{i}", tag="M")

            if i == 0:
                # M_0 rot = rj_0 (dims k,c map to row,col), t_0 = tf_0
                rj_classic(0, _ap(m_t[:, 0, 0, 0], [[4 * E, 3], [E, 3], [1, E]]))
                nc.sync.dma_start(
                    out=m_tr(m_t),
                    in_=bass.AP(tensor=cT_d.tensor, offset=cT_d.offset,
                                ap=[[0, P], [1, 3 * E]]))
            else:
                rj_dst = _ap(rja[:, i, 0, 0, 0], [[E, 3], [3 * E, 3], [1, E]])
                if i < 2:
                    rj_classic(i, rj_dst)
                else:
                    rj_fin(i, rj_dst)
                # prod[a, c', k] = R_{i-1}[a, k] * [Rj_i | tf_i][k, c']
                r_src = _ap(m_prev[:, 0, 0, 0],
                            [[4 * E, 3], [0, 4], [1, 3 * E]])
                rj_src = _ap(rja[:, i, 0, 0, 0], [[0, 3], [1, 12 * E]])
                nc.vector.tensor_tensor(prod[:], r_src, rj_src, MUL)
                # M = sum_k prod (two adds), then t += t_prev
                pk = [_ap(prod[:, 0, 0, k, 0],
                          [[12 * E, 3], [3 * E, 4], [1, E]])
                      for k in range(3)]
                nc.vector.tensor_tensor(m1[:], pk[0], pk[1], ADD)
                nc.vector.tensor_tensor(m_t[:], m1[:], pk[2], ADD)
                nc.vector.tensor_tensor(m_tr(m_t), m_tr(m_t), m_tr(m_prev),
                                        ADD)

            # output: [link, p, comp, e]; comp = 9 rot then 3 trans
            dst_r = bass.AP(tensor=out_d.tensor,
                            offset=out_d.offset + i * 12 * BC,
                            ap=[[12 * E, P], [E, 9], [1, E]])
            nc.sync.dma_start(
                out=dst_r,
                in_=_ap(m_t[:, 0, 0, 0], [[4 * E, 3], [E, 3], [1, E]]))
            dst_t = bass.AP(tensor=out_d.tensor,
                            offset=out_d.offset + i * 12 * BC + 9 * E,
                            ap=[[12 * E, P], [E, 3], [1, E]])
            nc.sync.dma_start(out=dst_t, in_=m_tr(m_t))
            m_prev = m_t


def build_module():
    nc = bacc.Bacc("TRN2", target_bir_lowering=False, debug=False,
                   enable_asserts=False, num_devices=N_CORES)
    q_d = nc.dram_tensor("q", [P, 12 * E], F16,
                         kind="ExternalInput").ap()
    cP_d = nc.dram_tensor("cP", [12 * 9 * EL], F16,
                          kind="ExternalInput").ap()
    bias_d = nc.dram_tensor("bias", [216], F32,
                            kind="ExternalInput").ap()
    bias2_d = nc.dram_tensor("bias2", [108], F32,
                             kind="ExternalInput").ap()
    c0_d = nc.dram_tensor("c0", [6 * 9 * EL], F16,
                          kind="ExternalInput").ap()
    cT_d = nc.dram_tensor("cT", [36 * E], F16,
                          kind="ExternalInput").ap()
    out_d = nc.dram_tensor("out", [N_LINKS, 12 * BC], F16,
                           kind="ExternalOutput").ap()
    with tile.TileContext(nc) as tc:
        _kernel_body(tc, out_d, q_d, cP_d, bias_d, bias2_d, c0_d, cT_d)
    nc.compile()
    nc.m = get_hw_module(nc.m)
    return nc


def make_consts(axes, rot_fixed, trans_fixed):
    """Host-side per-link constant prep (float64).

    Returns (cP, bias, cT):
      cP   — folded constant A - 1/2 - (B^2+C^2)/2, [12,9,EL] fp16
      bias — square biases (B*r then C*r per link, k-major), [216] fp32
      cT   — tf expanded over E, [12,3,E] fp16
    """
    ax = np.asarray(axes, np.float64)
    Rf = np.asarray(rot_fixed, np.float64)
    tf = np.asarray(trans_fixed, np.float64)
    A = np.zeros((N_LINKS, 3, 3))
    B = np.zeros((N_LINKS, 3, 3))
    C = np.zeros((N_LINKS, 3, 3))
    for i in range(N_LINKS):
        x, y, z = ax[i]
        K = np.array([[0.0, -z, y], [z, 0.0, -x], [-y, x, 0.0]])
        KK = K @ K
        A[i] = Rf[i] + Rf[i] @ KK
        B[i] = Rf[i] @ K
        C[i] = -(Rf[i] @ KK)

    cPf = A - 0.5 - 0.5 * (B * B + C * C)         # [12,3,3]
    bias2 = cPf.reshape(N_LINKS * 9)              # [108] (ACT-folded links)
    cP = np.repeat(cPf.reshape(N_LINKS, 9, 1), EL, axis=2)
    bias = np.concatenate(
        [np.concatenate([B[i].reshape(9), C[i].reshape(9)])
         for i in range(N_LINKS)]) * RHALF        # [216]
    c0 = np.stack([C[:2], B[:2], A[:2]])          # [3, 2, 3, 3]
    c0 = np.repeat(c0.reshape(6, 9, 1), EL, axis=2)
    tf_exp = np.repeat(tf.reshape(N_LINKS, 3, 1), E, axis=2)  # [i, k, E]
    return (cP.ravel().astype(np.float16), bias.astype(np.float32),
            bias2.astype(np.float32), c0.ravel().astype(np.float16),
            tf_exp.ravel().astype(np.float16))


_NC_CACHE = None


def get_module():
    global _NC_CACHE
    if _NC_CACHE is None:
        _NC_CACHE = build_module()
    return _NC_CACHE


def run(q, axes, rot_fixed, trans_fixed, trace=False):
    nc = get_module()
    cP, bias, bias2, c0, cT = make_consts(axes, rot_fixed, trans_fixed)
    # [B, 12] -> per core [P, 12, E] fp16 (batch-innermost),
    # range-reduced to [-pi, pi] (input preprocessing, like the cast)
    qf = np.asarray(q, np.float32)
    q16 = (qf - (2 * np.pi) * np.round(qf / (2 * np.pi))).astype(np.float16)
    q_sh = np.ascontiguousarray(
        q16.reshape(N_CORES, P, E, N_LINKS).transpose(0, 1, 3, 2)
    ).reshape(N_CORES, P, 12 * E)
    in_maps = [{"q": q_sh[i], "cP": cP, "bias": bias, "bias2": bias2,
                "c0": c0, "cT": cT}
               for i in range(N_CORES)]
    res = bass_utils.run_bass_kernel_spmd(
        nc, in_maps, core_ids=list(range(N_CORES)), trace=trace)
    # device out: [12 links, P, 12 comps, E] fp16, b = p*E + e
    out = np.empty((BATCH, N_LINKS, 12), np.float32)
    for i, r in enumerate(res.results):
        dev = r["out"].reshape(N_LINKS, P, 12, E)
        out[i * BC:(i + 1) * BC] = (
            dev.transpose(1, 3, 0, 2).reshape(BC, N_LINKS, 12)
            .astype(np.float32))
    return out, res


def kernel(q, axes, rot_fixed, trans_fixed):
    out, _ = run(q, axes, rot_fixed, trans_fixed, trace=False)
    return out


# revision 32
# speedup vs baseline: 1.6834x; 1.0162x over previous
"""Trainium2 Bass kernel: batched serial-chain forward kinematics (fp16).

Problem: nn_DifferentiableRobotModel — q [262144, 12] joint angles,
per-link constant transforms. Output [B, 12, 12] = per link
(flattened 3x3 rot, 3 trans).

Math (per batch element b, per link i, sequential over i):
    Rj_i = A_i + sin(q_i) * B_i + cos(q_i) * C_i     (3x3)
    R_i  = R_{i-1} @ Rj_i        (R_{-1} = I)
    t_i  = t_{i-1} + R_{i-1} @ tf_i   (t_{-1} = 0)
with host-precomputed per-link constants A/B/C (Rodrigues expansion of
the fixed transform times the joint rotation), tf_i = trans_fixed_i.

Device strategy: pure data parallel over 8 cores (batch split); fp16,
batch-innermost layout [..., E=256]; every DVE op runs in 2x_1P mode.
The DVE is the bottleneck engine, so the rj build is split with the
otherwise-idle ACT engine via the square identity (s^2 + c^2 = 1):

    s*B_kc + c*C_kc = (s*r + B_kc*r)^2 + (c*r + C_kc*r)^2 - (B^2+C^2)/2
                      - 1/2,     r = sqrt(1/2)

ACT's Square activation computes each (s*r + bias)^2 with the per-entry
bias as a runtime [P,1] scalar, so the DVE's per-link rj work drops
from 4 ops/36E to 2 adds/18E (u+v, then + a folded constant). The
GpSimd engine is left idle on purpose: concurrent GpSimd tensor ops
steal DVE SBUF ports (~4x DVE slowdown, measured). sin/cos run on ACT
in link-groups; q is range-reduced to [-pi, pi] and cast to fp16 on
the host (input preprocessing; the ACT Sin spline is only valid there,
measured). Per-link M is stored interleaved [row, 4, E] = [R row | t]
so the k-reduction is two 12E adds. Output DMAs ride the idle sync
queue as fp16 (9E rot + 3E trans per link) and are transposed/upcast
to fp32 on the host (rel err ~3e-3, inside the 2e-2 gate).
"""

import math

import numpy as np

import concourse.bass as bass
import concourse.bacc as bacc
import concourse.mybir as mybir
import concourse.tile as tile
from concourse import bass_utils
from concourse.bass_interp import get_hw_module

N_CORES = 8
N_LINKS = 12
BATCH = 262144
BC = BATCH // N_CORES          # batch per core
P = 128                        # SBUF partitions
E = BC // P                    # batch elems per partition (256)
EL = 8                         # const expansion width (innermost run)
EH = E // EL
GS = (1, 1, 2, 2, 3, 3)        # trig pipeline group sizes
GOF = (0, 1, 2, 4, 6, 9)       # group start links
RHALF = math.sqrt(0.5)

F16 = mybir.dt.float16
F32 = mybir.dt.float32
MUL = mybir.AluOpType.mult
ADD = mybir.AluOpType.add
SIN = mybir.ActivationFunctionType.Sin
ABS = mybir.ActivationFunctionType.Abs
SQR = mybir.ActivationFunctionType.Square
IDN = mybir.ActivationFunctionType.Identity


def _ap(sl, dims):
    """New AP from slice `sl` keeping its partition dim + given free dims."""
    return bass.AP(tensor=sl.tensor, offset=sl.offset,
                   ap=[list(sl.ap[0])] + [list(d) for d in dims])


def _grp(i):
    for g in range(5, -1, -1):
        if i >= GOF[g]:
            return g, i - GOF[g]


def _kernel_body(tc, out_d, q_d, cP_d, bias_d, bias2_d, c0_d, cT_d):
    nc = tc.nc
    with (
        tc.tile_pool(name="io", bufs=1) as io,
        tc.tile_pool(name="mm", bufs=4) as mm,
        tc.tile_pool(name="up", bufs=4) as up,
        tc.tile_pool(name="vp", bufs=4) as vp,
        tc.tile_pool(name="wk", bufs=1) as wk,
    ):
        rja = wk.tile([P, N_LINKS, 4, 3, E], F16, tag="rja")
        q16 = io.tile([P, N_LINKS, E], F16, tag="q")
        cst = io.tile([P, 12 * 9 * EL], F16, tag="cst")   # folded const
        bia = io.tile([P, 216], F32, tag="bia")           # square biases
        bia2 = io.tile([P, 108], F32, tag="bia2")         # folded-const biases
        cl0 = io.tile([P, 6 * 9 * EL], F16, tag="cl0")    # C,B,A links 0,1

        # ---- inputs: link-0/1 classic consts ride the scalar queue (it
        # is idle until the ACT table load finishes), q group 0 leads the
        # sync ring since it gates trig; tf (E-expanded) follows.
        def rep_in(d, n):
            return bass.AP(tensor=d.tensor, offset=d.offset, ap=[[0, P], [1, n]])

        nc.scalar.dma_start(out=cl0[:], in_=rep_in(c0_d, 6 * 9 * EL))

        # dummy self-referential Sin: triggers the ACT table load
        warm = wk.tile([P, 1], F32, tag="warm")
        nc.scalar.activation(warm[:], warm[:], SIN)

        nc.sync.dma_start(out=q16[:, 0, :],
                          in_=bass.AP(tensor=q_d.tensor, offset=q_d.offset,
                                      ap=[[12 * E, P], [1, E]]))
        nc.sync.dma_start(out=q16[:, 1:, :],
                          in_=bass.AP(tensor=q_d.tensor, offset=q_d.offset + E,
                                      ap=[[12 * E, P], [1, 11 * E]]))
        nc.sync.dma_start(out=bia[:], in_=rep_in(bias_d, 216))
        nc.sync.dma_start(out=bia2[:], in_=rep_in(bias2_d, 108))
        nc.sync.dma_start(out=cst[:], in_=rep_in(cP_d, 12 * 9 * EL))
        for lo, hi in ((0, 4), (4, 8), (8, 12)):
            tf_dst = _ap(rja[:, lo, 3, 0, 0],
                         [[12 * E, hi - lo], [1, 3 * E]])
            nc.sync.dma_start(
                out=tf_dst,
                in_=bass.AP(tensor=cT_d.tensor, offset=cT_d.offset + lo * 3 * E,
                            ap=[[0, P], [1, (hi - lo) * 3 * E]]))

        # ---- ACT: per group sin/cos (in-place abs trick), then per link
        # 18 Square ops producing u_i = (s*r + B*r)^2, v_i = (c*r + C*r)^2
        hpi = wk.tile([P, 1], F32, tag="hpi")
        nc.vector.memset(hpi[:], math.pi / 2)
        s16 = [wk.tile([P, GS[g], E], F16, name=f"s{g}", tag=f"s{g}")
               for g in range(6)]
        c16 = [wk.tile([P, GS[g], E], F16, name=f"cc{g}", tag=f"cc{g}")
               for g in range(6)]
        uv = {}
        for g in range(6):
            q_sl = q16[:, GOF[g]:GOF[g] + GS[g], :]
            nc.scalar.activation(c16[g][:], q_sl, ABS)
            nc.scalar.activation(c16[g][:], c16[g][:], SIN,
                                 bias=hpi[:], scale=-1.0)
            nc.scalar.activation(s16[g][:], q_sl, SIN)
            for j in range(GS[g]):
                i = GOF[g] + j
                if i < 2:
                    continue        # links 0,1: classic DVE build
                u = up.tile([P, 9, E], F16, name=f"u{i}", tag="u")
                v = vp.tile([P, 9, E], F16, name=f"v{i}", tag="v")
                uv[i] = (u, v)
                s_sl = s16[g][:, j, :]
                c_sl = c16[g][:, j, :]
                for kc in range(9):
                    nc.scalar.activation(u[:, kc, :], s_sl, SQR,
                                         bias=bia[:, i * 18 + kc:i * 18 + kc + 1],
                                         scale=RHALF)
                    nc.scalar.activation(v[:, kc, :], c_sl, SQR,
                                         bias=bia[:, i * 18 + 9 + kc:i * 18 + 10 + kc],
                                         scale=RHALF)
                if i >= 6:
                    # fold the per-entry constant into u on ACT (no DVE dep)
                    for kc in range(9):
                        nc.scalar.activation(
                            u[:, kc, :], u[:, kc, :], IDN,
                            bias=bia2[:, i * 9 + kc:i * 9 + kc + 1])

        # ---- DVE: per link rj finish (2 adds), chain step; output on sync
        def cst_bc(i):                  # folded const, bcast over EH
            return _ap(cst[:, i * 72],
                       [[3 * EL, 3], [EL, 3], [0, EH], [1, EL]])

        def rj_fin(i, dst):
            # dst[k,c] = u_i[kc] + v_i[kc] (+ cstP_i[kc] unless pre-folded)
            u, v = uv[i]
            usr = _ap(u[:, 0, 0], [[3 * E, 3], [E, 3], [1, E]])
            vsr = _ap(v[:, 0, 0], [[3 * E, 3], [E, 3], [1, E]])
            nc.vector.tensor_tensor(dst, usr, vsr, ADD)
            if i < 6:
                nc.vector.tensor_tensor(dst, dst, cst_bc(i), ADD)

        def sc_bc(t, i):                # s/c bcast over (k, c) outermost
            g, j = _grp(i)
            return _ap(t[g][:, j, 0], [[0, 3], [0, 3], [1, E]])

        def cl0_bc(mat, i):             # classic C/B/A consts, links 0..1
            off = (mat * 2 + i) * 72
            return _ap(cl0[:, off],
                       [[3 * EL, 3], [EL, 3], [0, EH], [1, EL]])

        w0 = wk.tile([P, 9, E], F16, tag="w0")
        w0f = _ap(w0[:, 0, 0], [[1, 9 * E]])

        def rj_classic(i, dst):
            # dst[k,c] = A + s*B + c*C; s-mult first (sin q retires before
            # the cos compose on ACT)
            nc.vector.tensor_tensor(w0f, sc_bc(s16, i), cl0_bc(1, i), MUL)
            nc.vector.tensor_tensor(dst, sc_bc(c16, i), cl0_bc(0, i), MUL)
            nc.vector.tensor_tensor(dst, dst, w0f, ADD)
            nc.vector.tensor_tensor(dst, dst, cl0_bc(2, i), ADD)

        prod = wk.tile([P, 3, 4, 3, E], F16, tag="prod")   # [a, c', k, e]
        m1 = wk.tile([P, 3, 4, E], F16, tag="m1")          # [a, c', e]

        def m_tr(m):                    # translation column of M: dims (a, e)
            return _ap(m[:, 0, 3, 0], [[4 * E, 3], [1, E]])

        m_prev = None
        for i in range(N_LINKS):
            m_t = mm.tile([P, 3, 4, E], F16, name=f"M{i}", tag="M")

            if i == 0:
                # M_0 rot = rj_0 (dims k,c map to row,col), t_0 = tf_0
                rj_classic(0, _ap(m_t[:, 0, 0, 0], [[4 * E, 3], [E, 3], [1, E]]))
                nc.sync.dma_start(
                    out=m_tr(m_t),
                    in_=bass.AP(tensor=cT_d.tensor, offset=cT_d.offset,
                                ap=[[0, P], [1, 3 * E]]))
            else:
                rj_dst = _ap(rja[:, i, 0, 0, 0], [[E, 3], [3 * E, 3], [1, E]])
                if i < 2:
                    rj_classic(i, rj_dst)
                else:
                    rj_fin(i, rj_dst)
                # prod[a, c', k] = R_{i-1}[a, k] * [Rj_i | tf_i][k, c']
                r_src = _ap(m_prev[:, 0, 0, 0],
                            [[4 * E, 3], [0, 4], [1, 3 * E]])
                rj_src = _ap(rja[:, i, 0, 0, 0], [[0, 3], [1, 12 * E]])
                nc.vector.tensor_tensor(prod[:], r_src, rj_src, MUL)
                # M = sum_k prod (two adds), then t += t_prev
                pk = [_ap(prod[:, 0, 0, k, 0],
                          [[12 * E, 3], [3 * E, 4], [1, E]])
                      for k in range(3)]
                nc.vector.tensor_tensor(m1[:], pk[0], pk[1], ADD)
                nc.vector.tensor_tensor(m_t[:], m1[:], pk[2], ADD)
                nc.vector.tensor_tensor(m_tr(m_t), m_tr(m_t), m_tr(m_prev),
                                        ADD)

            # output: [link, p, comp, e]; comp = 9 rot then 3 trans
            dst_r = bass.AP(tensor=out_d.tensor,
                            offset=out_d.offset + i * 12 * BC,
                            ap=[[12 * E, P], [E, 9], [1, E]])
            nc.sync.dma_start(
                out=dst_r,
                in_=_ap(m_t[:, 0, 0, 0], [[4 * E, 3], [E, 3], [1, E]]))
            dst_t = bass.AP(tensor=out_d.tensor,
                            offset=out_d.offset + i * 12 * BC + 9 * E,
                            ap=[[12 * E, P], [E, 3], [1, E]])
            # last link: t rides the scalar queue so it overlaps the R DMA
            eng_t = nc.scalar if i == N_LINKS - 1 else nc.sync
            eng_t.dma_start(out=dst_t, in_=m_tr(m_t))
            m_prev = m_t


def build_module():
    nc = bacc.Bacc("TRN2", target_bir_lowering=False, debug=False,
                   enable_asserts=False, num_devices=N_CORES)
    q_d = nc.dram_tensor("q", [P, 12 * E], F16,
                         kind="ExternalInput").ap()
    cP_d = nc.dram_tensor("cP", [12 * 9 * EL], F16,
                          kind="ExternalInput").ap()
    bias_d = nc.dram_tensor("bias", [216], F32,
                            kind="ExternalInput").ap()
    bias2_d = nc.dram_tensor("bias2", [108], F32,
                             kind="ExternalInput").ap()
    c0_d = nc.dram_tensor("c0", [6 * 9 * EL], F16,
                          kind="ExternalInput").ap()
    cT_d = nc.dram_tensor("cT", [36 * E], F16,
                          kind="ExternalInput").ap()
    out_d = nc.dram_tensor("out", [N_LINKS, 12 * BC], F16,
                           kind="ExternalOutput").ap()
    with tile.TileContext(nc) as tc:
        _kernel_body(tc, out_d, q_d, cP_d, bias_d, bias2_d, c0_d, cT_d)
    nc.compile()
    nc.m = get_hw_module(nc.m)
    return nc


def make_consts(axes, rot_fixed, trans_fixed):
    """Host-side per-link constant prep (float64).

    Returns (cP, bias, cT):
      cP   — folded constant A - 1/2 - (B^2+C^2)/2, [12,9,EL] fp16
      bias — square biases (B*r then C*r per link, k-major), [216] fp32
      cT   — tf expanded over E, [12,3,E] fp16
    """
    ax = np.asarray(axes, np.float64)
    Rf = np.asarray(rot_fixed, np.float64)
    tf = np.asarray(trans_fixed, np.float64)
    A = np.zeros((N_LINKS, 3, 3))
    B = np.zeros((N_LINKS, 3, 3))
    C = np.zeros((N_LINKS, 3, 3))
    for i in range(N_LINKS):
        x, y, z = ax[i]
        K = np.array([[0.0, -z, y], [z, 0.0, -x], [-y, x, 0.0]])
        KK = K @ K
        A[i] = Rf[i] + Rf[i] @ KK
        B[i] = Rf[i] @ K
        C[i] = -(Rf[i] @ KK)

    cPf = A - 0.5 - 0.5 * (B * B + C * C)         # [12,3,3]
    bias2 = cPf.reshape(N_LINKS * 9)              # [108] (ACT-folded links)
    cP = np.repeat(cPf.reshape(N_LINKS, 9, 1), EL, axis=2)
    bias = np.concatenate(
        [np.concatenate([B[i].reshape(9), C[i].reshape(9)])
         for i in range(N_LINKS)]) * RHALF        # [216]
    c0 = np.stack([C[:2], B[:2], A[:2]])          # [3, 2, 3, 3]
    c0 = np.repeat(c0.reshape(6, 9, 1), EL, axis=2)
    tf_exp = np.repeat(tf.reshape(N_LINKS, 3, 1), E, axis=2)  # [i, k, E]
    return (cP.ravel().astype(np.float16), bias.astype(np.float32),
            bias2.astype(np.float32), c0.ravel().astype(np.float16),
            tf_exp.ravel().astype(np.float16))


_NC_CACHE = None


def get_module():
    global _NC_CACHE
    if _NC_CACHE is None:
        _NC_CACHE = build_module()
    return _NC_CACHE


def run(q, axes, rot_fixed, trans_fixed, trace=False):
    nc = get_module()
    cP, bias, bias2, c0, cT = make_consts(axes, rot_fixed, trans_fixed)
    # [B, 12] -> per core [P, 12, E] fp16 (batch-innermost),
    # range-reduced to [-pi, pi] (input preprocessing, like the cast)
    qf = np.asarray(q, np.float32)
    q16 = (qf - (2 * np.pi) * np.round(qf / (2 * np.pi))).astype(np.float16)
    q_sh = np.ascontiguousarray(
        q16.reshape(N_CORES, P, E, N_LINKS).transpose(0, 1, 3, 2)
    ).reshape(N_CORES, P, 12 * E)
    in_maps = [{"q": q_sh[i], "cP": cP, "bias": bias, "bias2": bias2,
                "c0": c0, "cT": cT}
               for i in range(N_CORES)]
    res = bass_utils.run_bass_kernel_spmd(
        nc, in_maps, core_ids=list(range(N_CORES)), trace=trace)
    # device out: [12 links, P, 12 comps, E] fp16, b = p*E + e
    out = np.empty((BATCH, N_LINKS, 12), np.float32)
    for i, r in enumerate(res.results):
        dev = r["out"].reshape(N_LINKS, P, 12, E)
        out[i * BC:(i + 1) * BC] = (
            dev.transpose(1, 3, 0, 2).reshape(BC, N_LINKS, 12)
            .astype(np.float32))
    return out, res


def kernel(q, axes, rot_fixed, trans_fixed):
    out, _ = run(q, axes, rot_fixed, trans_fixed, trace=False)
    return out


# revision 34
# speedup vs baseline: 1.6856x; 1.0013x over previous
"""Trainium2 Bass kernel: batched serial-chain forward kinematics (fp16).

Problem: nn_DifferentiableRobotModel — q [262144, 12] joint angles,
per-link constant transforms. Output [B, 12, 12] = per link
(flattened 3x3 rot, 3 trans).

Math (per batch element b, per link i, sequential over i):
    Rj_i = A_i + sin(q_i) * B_i + cos(q_i) * C_i     (3x3)
    R_i  = R_{i-1} @ Rj_i        (R_{-1} = I)
    t_i  = t_{i-1} + R_{i-1} @ tf_i   (t_{-1} = 0)
with host-precomputed per-link constants A/B/C (Rodrigues expansion of
the fixed transform times the joint rotation), tf_i = trans_fixed_i.

Device strategy: pure data parallel over 8 cores (batch split); fp16,
batch-innermost layout [..., E=256]; every DVE op runs in 2x_1P mode.
The DVE is the bottleneck engine, so the rj build is split with the
otherwise-idle ACT engine via the square identity (s^2 + c^2 = 1):

    s*B_kc + c*C_kc = (s*r + B_kc*r)^2 + (c*r + C_kc*r)^2 - (B^2+C^2)/2
                      - 1/2,     r = sqrt(1/2)

ACT's Square activation computes each (s*r + bias)^2 with the per-entry
bias as a runtime [P,1] scalar, so the DVE's per-link rj work drops
from 4 ops/36E to 2 adds/18E (u+v, then + a folded constant). The
GpSimd engine is left idle on purpose: concurrent GpSimd tensor ops
steal DVE SBUF ports (~4x DVE slowdown, measured). sin/cos run on ACT
in link-groups; q is range-reduced to [-pi, pi] and cast to fp16 on
the host (input preprocessing; the ACT Sin spline is only valid there,
measured). Per-link M is stored interleaved [row, 4, E] = [R row | t]
so the k-reduction is two 12E adds. Output DMAs ride the idle sync
queue as fp16 (9E rot + 3E trans per link) and are transposed/upcast
to fp32 on the host (rel err ~3e-3, inside the 2e-2 gate).
"""

import math

import numpy as np

import concourse.bass as bass
import concourse.bacc as bacc
import concourse.mybir as mybir
import concourse.tile as tile
from concourse import bass_utils
from concourse.bass_interp import get_hw_module

N_CORES = 8
N_LINKS = 12
BATCH = 262144
BC = BATCH // N_CORES          # batch per core
P = 128                        # SBUF partitions
E = BC // P                    # batch elems per partition (256)
EL = 8                         # const expansion width (innermost run)
EH = E // EL
GS = (1, 1, 2, 2, 3, 3)        # trig pipeline group sizes
GOF = (0, 1, 2, 4, 6, 9)       # group start links
RHALF = math.sqrt(0.5)

F16 = mybir.dt.float16
F32 = mybir.dt.float32
MUL = mybir.AluOpType.mult
ADD = mybir.AluOpType.add
SIN = mybir.ActivationFunctionType.Sin
ABS = mybir.ActivationFunctionType.Abs
SQR = mybir.ActivationFunctionType.Square
IDN = mybir.ActivationFunctionType.Identity


def _ap(sl, dims):
    """New AP from slice `sl` keeping its partition dim + given free dims."""
    return bass.AP(tensor=sl.tensor, offset=sl.offset,
                   ap=[list(sl.ap[0])] + [list(d) for d in dims])


def _grp(i):
    for g in range(5, -1, -1):
        if i >= GOF[g]:
            return g, i - GOF[g]


def _kernel_body(tc, out_d, q_d, cP_d, bias_d, bias2_d, c0_d, cT_d):
    nc = tc.nc
    with (
        tc.tile_pool(name="io", bufs=1) as io,
        tc.tile_pool(name="mm", bufs=4) as mm,
        tc.tile_pool(name="up", bufs=4) as up,
        tc.tile_pool(name="vp", bufs=4) as vp,
        tc.tile_pool(name="wk", bufs=1) as wk,
    ):
        rja = wk.tile([P, N_LINKS, 4, 3, E], F16, tag="rja")
        q16 = io.tile([P, N_LINKS, E], F16, tag="q")
        cst = io.tile([P, 12 * 9 * EL], F16, tag="cst")   # folded const
        bia = io.tile([P, 216], F32, tag="bia")           # square biases
        bia2 = io.tile([P, 108], F32, tag="bia2")         # folded-const biases
        cl0 = io.tile([P, 6 * 9 * EL], F16, tag="cl0")    # C,B,A links 0,1

        # ---- inputs: link-0/1 classic consts ride the scalar queue (it
        # is idle until the ACT table load finishes), q group 0 leads the
        # sync ring since it gates trig; tf (E-expanded) follows.
        def rep_in(d, n):
            return bass.AP(tensor=d.tensor, offset=d.offset, ap=[[0, P], [1, n]])

        nc.scalar.dma_start(out=cl0[:], in_=rep_in(c0_d, 6 * 9 * EL))

        # dummy self-referential Sin: triggers the ACT table load
        warm = wk.tile([P, 1], F32, tag="warm")
        nc.scalar.activation(warm[:], warm[:], SIN)

        nc.sync.dma_start(out=q16[:, 0, :],
                          in_=bass.AP(tensor=q_d.tensor, offset=q_d.offset,
                                      ap=[[12 * E, P], [1, E]]))
        nc.sync.dma_start(out=q16[:, 1:, :],
                          in_=bass.AP(tensor=q_d.tensor, offset=q_d.offset + E,
                                      ap=[[12 * E, P], [1, 11 * E]]))
        nc.sync.dma_start(out=bia[:], in_=rep_in(bias_d, 216))
        nc.sync.dma_start(out=bia2[:], in_=rep_in(bias2_d, 108))
        nc.sync.dma_start(out=cst[:], in_=rep_in(cP_d, 12 * 9 * EL))
        for lo, hi in ((0, 4), (4, 8), (8, 12)):
            tf_dst = _ap(rja[:, lo, 3, 0, 0],
                         [[12 * E, hi - lo], [1, 3 * E]])
            nc.sync.dma_start(
                out=tf_dst,
                in_=bass.AP(tensor=cT_d.tensor, offset=cT_d.offset + lo * 3 * E,
                            ap=[[0, P], [1, (hi - lo) * 3 * E]]))

        # ---- ACT: per group sin/cos (in-place abs trick), then per link
        # 18 Square ops producing u_i = (s*r + B*r)^2, v_i = (c*r + C*r)^2
        hpi = wk.tile([P, 1], F32, tag="hpi")
        nc.vector.memset(hpi[:], math.pi / 2)
        s16 = [wk.tile([P, GS[g], E], F16, name=f"s{g}", tag=f"s{g}")
               for g in range(6)]
        c16 = [wk.tile([P, GS[g], E], F16, name=f"cc{g}", tag=f"cc{g}")
               for g in range(6)]
        uv = {}
        for g in range(6):
            q_sl = q16[:, GOF[g]:GOF[g] + GS[g], :]
            nc.scalar.activation(c16[g][:], q_sl, ABS)
            nc.scalar.activation(c16[g][:], c16[g][:], SIN,
                                 bias=hpi[:], scale=-1.0)
            nc.scalar.activation(s16[g][:], q_sl, SIN)
            for j in range(GS[g]):
                i = GOF[g] + j
                if i < 2:
                    continue        # links 0,1: classic DVE build
                u = up.tile([P, 9, E], F16, name=f"u{i}", tag="u")
                v = vp.tile([P, 9, E], F16, name=f"v{i}", tag="v")
                uv[i] = (u, v)
                s_sl = s16[g][:, j, :]
                c_sl = c16[g][:, j, :]
                for kc in range(9):
                    nc.scalar.activation(u[:, kc, :], s_sl, SQR,
                                         bias=bia[:, i * 18 + kc:i * 18 + kc + 1],
                                         scale=RHALF)
                    nc.scalar.activation(v[:, kc, :], c_sl, SQR,
                                         bias=bia[:, i * 18 + 9 + kc:i * 18 + 10 + kc],
                                         scale=RHALF)
                if i >= 5:
                    # fold the per-entry constant into u on ACT (no DVE dep)
                    for kc in range(9):
                        nc.scalar.activation(
                            u[:, kc, :], u[:, kc, :], IDN,
                            bias=bia2[:, i * 9 + kc:i * 9 + kc + 1])

        # ---- DVE: per link rj finish (2 adds), chain step; output on sync
        def cst_bc(i):                  # folded const, bcast over EH
            return _ap(cst[:, i * 72],
                       [[3 * EL, 3], [EL, 3], [0, EH], [1, EL]])

        def rj_fin(i, dst):
            # dst[k,c] = u_i[kc] + v_i[kc] (+ cstP_i[kc] unless pre-folded)
            u, v = uv[i]
            usr = _ap(u[:, 0, 0], [[3 * E, 3], [E, 3], [1, E]])
            vsr = _ap(v[:, 0, 0], [[3 * E, 3], [E, 3], [1, E]])
            nc.vector.tensor_tensor(dst, usr, vsr, ADD)
            if i < 5:
                nc.vector.tensor_tensor(dst, dst, cst_bc(i), ADD)

        def sc_bc(t, i):                # s/c bcast over (k, c) outermost
            g, j = _grp(i)
            return _ap(t[g][:, j, 0], [[0, 3], [0, 3], [1, E]])

        def cl0_bc(mat, i):             # classic C/B/A consts, links 0..1
            off = (mat * 2 + i) * 72
            return _ap(cl0[:, off],
                       [[3 * EL, 3], [EL, 3], [0, EH], [1, EL]])

        w0 = wk.tile([P, 9, E], F16, tag="w0")
        w0f = _ap(w0[:, 0, 0], [[1, 9 * E]])

        def rj_classic(i, dst):
            # dst[k,c] = A + s*B + c*C; s-mult first (sin q retires before
            # the cos compose on ACT)
            nc.vector.tensor_tensor(w0f, sc_bc(s16, i), cl0_bc(1, i), MUL)
            nc.vector.tensor_tensor(dst, sc_bc(c16, i), cl0_bc(0, i), MUL)
            nc.vector.tensor_tensor(dst, dst, w0f, ADD)
            nc.vector.tensor_tensor(dst, dst, cl0_bc(2, i), ADD)

        prod = wk.tile([P, 3, 4, 3, E], F16, tag="prod")   # [a, c', k, e]
        m1 = wk.tile([P, 3, 4, E], F16, tag="m1")          # [a, c', e]

        def m_tr(m):                    # translation column of M: dims (a, e)
            return _ap(m[:, 0, 3, 0], [[4 * E, 3], [1, E]])

        m_prev = None
        for i in range(N_LINKS):
            m_t = mm.tile([P, 3, 4, E], F16, name=f"M{i}", tag="M")

            if i == 0:
                # M_0 rot = rj_0 (dims k,c map to row,col), t_0 = tf_0
                rj_classic(0, _ap(m_t[:, 0, 0, 0], [[4 * E, 3], [E, 3], [1, E]]))
                nc.sync.dma_start(
                    out=m_tr(m_t),
                    in_=bass.AP(tensor=cT_d.tensor, offset=cT_d.offset,
                                ap=[[0, P], [1, 3 * E]]))
            else:
                rj_dst = _ap(rja[:, i, 0, 0, 0], [[E, 3], [3 * E, 3], [1, E]])
                if i < 2:
                    rj_classic(i, rj_dst)
                else:
                    rj_fin(i, rj_dst)
                # prod[a, c', k] = R_{i-1}[a, k] * [Rj_i | tf_i][k, c']
                r_src = _ap(m_prev[:, 0, 0, 0],
                            [[4 * E, 3], [0, 4], [1, 3 * E]])
                rj_src = _ap(rja[:, i, 0, 0, 0], [[0, 3], [1, 12 * E]])
                nc.vector.tensor_tensor(prod[:], r_src, rj_src, MUL)
                # M = sum_k prod (two adds), then t += t_prev
                pk = [_ap(prod[:, 0, 0, k, 0],
                          [[12 * E, 3], [3 * E, 4], [1, E]])
                      for k in range(3)]
                nc.vector.tensor_tensor(m1[:], pk[0], pk[1], ADD)
                nc.vector.tensor_tensor(m_t[:], m1[:], pk[2], ADD)
                nc.vector.tensor_tensor(m_tr(m_t), m_tr(m_t), m_tr(m_prev),
                                        ADD)

            # output: [link, p, comp, e]; comp = 9 rot then 3 trans
            dst_r = bass.AP(tensor=out_d.tensor,
                            offset=out_d.offset + i * 12 * BC,
                            ap=[[12 * E, P], [E, 9], [1, E]])
            nc.sync.dma_start(
                out=dst_r,
                in_=_ap(m_t[:, 0, 0, 0], [[4 * E, 3], [E, 3], [1, E]]))
            dst_t = bass.AP(tensor=out_d.tensor,
                            offset=out_d.offset + i * 12 * BC + 9 * E,
                            ap=[[12 * E, P], [E, 3], [1, E]])
            # last link: t rides the scalar queue so it overlaps the R DMA
            eng_t = nc.scalar if i == N_LINKS - 1 else nc.sync
            eng_t.dma_start(out=dst_t, in_=m_tr(m_t))
            m_prev = m_t


def build_module():
    nc = bacc.Bacc("TRN2", target_bir_lowering=False, debug=False,
                   enable_asserts=False, num_devices=N_CORES)
    q_d = nc.dram_tensor("q", [P, 12 * E], F16,
                         kind="ExternalInput").ap()
    cP_d = nc.dram_tensor("cP", [12 * 9 * EL], F16,
                          kind="ExternalInput").ap()
    bias_d = nc.dram_tensor("bias", [216], F32,
                            kind="ExternalInput").ap()
    bias2_d = nc.dram_tensor("bias2", [108], F32,
                             kind="ExternalInput").ap()
    c0_d = nc.dram_tensor("c0", [6 * 9 * EL], F16,
                          kind="ExternalInput").ap()
    cT_d = nc.dram_tensor("cT", [36 * E], F16,
                          kind="ExternalInput").ap()
    out_d = nc.dram_tensor("out", [N_LINKS, 12 * BC], F16,
                           kind="ExternalOutput").ap()
    with tile.TileContext(nc) as tc:
        _kernel_body(tc, out_d, q_d, cP_d, bias_d, bias2_d, c0_d, cT_d)
    nc.compile()
    nc.m = get_hw_module(nc.m)
    return nc


def make_consts(axes, rot_fixed, trans_fixed):
    """Host-side per-link constant prep (float64).

    Returns (cP, bias, cT):
      cP   — folded constant A - 1/2 - (B^2+C^2)/2, [12,9,EL] fp16
      bias — square biases (B*r then C*r per link, k-major), [216] fp32
      cT   — tf expanded over E, [12,3,E] fp16
    """
    ax = np.asarray(axes, np.float64)
    Rf = np.asarray(rot_fixed, np.float64)
    tf = np.asarray(trans_fixed, np.float64)
    A = np.zeros((N_LINKS, 3, 3))
    B = np.zeros((N_LINKS, 3, 3))
    C = np.zeros((N_LINKS, 3, 3))
    for i in range(N_LINKS):
        x, y, z = ax[i]
        K = np.array([[0.0, -z, y], [z, 0.0, -x], [-y, x, 0.0]])
        KK = K @ K
        A[i] = Rf[i] + Rf[i] @ KK
        B[i] = Rf[i] @ K
        C[i] = -(Rf[i] @ KK)

    cPf = A - 0.5 - 0.5 * (B * B + C * C)         # [12,3,3]
    bias2 = cPf.reshape(N_LINKS * 9)              # [108] (ACT-folded links)
    cP = np.repeat(cPf.reshape(N_LINKS, 9, 1), EL, axis=2)
    bias = np.concatenate(
        [np.concatenate([B[i].reshape(9), C[i].reshape(9)])
         for i in range(N_LINKS)]) * RHALF        # [216]
    c0 = np.stack([C[:2], B[:2], A[:2]])          # [3, 2, 3, 3]
    c0 = np.repeat(c0.reshape(6, 9, 1), EL, axis=2)
    tf_exp = np.repeat(tf.reshape(N_LINKS, 3, 1), E, axis=2)  # [i, k, E]
    return (cP.ravel().astype(np.float16), bias.astype(np.float32),
            bias2.astype(np.float32), c0.ravel().astype(np.float16),
            tf_exp.ravel().astype(np.float16))


_NC_CACHE = None


def get_module():
    global _NC_CACHE
    if _NC_CACHE is None:
        _NC_CACHE = build_module()
    return _NC_CACHE


def run(q, axes, rot_fixed, trans_fixed, trace=False):
    nc = get_module()
    cP, bias, bias2, c0, cT = make_consts(axes, rot_fixed, trans_fixed)
    # [B, 12] -> per core [P, 12, E] fp16 (batch-innermost),
    # range-reduced to [-pi, pi] (input preprocessing, like the cast)
    qf = np.asarray(q, np.float32)
    q16 = (qf - (2 * np.pi) * np.round(qf / (2 * np.pi))).astype(np.float16)
    q_sh = np.ascontiguousarray(
        q16.reshape(N_CORES, P, E, N_LINKS).transpose(0, 1, 3, 2)
    ).reshape(N_CORES, P, 12 * E)
    in_maps = [{"q": q_sh[i], "cP": cP, "bias": bias, "bias2": bias2,
                "c0": c0, "cT": cT}
               for i in range(N_CORES)]
    res = bass_utils.run_bass_kernel_spmd(
        nc, in_maps, core_ids=list(range(N_CORES)), trace=trace)
    # device out: [12 links, P, 12 comps, E] fp16, b = p*E + e
    out = np.empty((BATCH, N_LINKS, 12), np.float32)
    for i, r in enumerate(res.results):
        dev = r["out"].reshape(N_LINKS, P, 12, E)
        out[i * BC:(i + 1) * BC] = (
            dev.transpose(1, 3, 0, 2).reshape(BC, N_LINKS, 12)
            .astype(np.float32))
    return out, res


def kernel(q, axes, rot_fixed, trans_fixed):
    out, _ = run(q, axes, rot_fixed, trans_fixed, trace=False)
    return out


# revision 36
# speedup vs baseline: 1.6902x; 1.0027x over previous
"""Trainium2 Bass kernel: batched serial-chain forward kinematics (fp16).

Problem: nn_DifferentiableRobotModel — q [262144, 12] joint angles,
per-link constant transforms. Output [B, 12, 12] = per link
(flattened 3x3 rot, 3 trans).

Math (per batch element b, per link i, sequential over i):
    Rj_i = A_i + sin(q_i) * B_i + cos(q_i) * C_i     (3x3)
    R_i  = R_{i-1} @ Rj_i        (R_{-1} = I)
    t_i  = t_{i-1} + R_{i-1} @ tf_i   (t_{-1} = 0)
with host-precomputed per-link constants A/B/C (Rodrigues expansion of
the fixed transform times the joint rotation), tf_i = trans_fixed_i.

Device strategy: pure data parallel over 8 cores (batch split); fp16,
batch-innermost layout [..., E=256]; every DVE op runs in 2x_1P mode.
The DVE is the bottleneck engine, so the rj build is split with the
otherwise-idle ACT engine via the square identity (s^2 + c^2 = 1):

    s*B_kc + c*C_kc = (s*r + B_kc*r)^2 + (c*r + C_kc*r)^2 - (B^2+C^2)/2
                      - 1/2,     r = sqrt(1/2)

ACT's Square activation computes each (s*r + bias)^2 with the per-entry
bias as a runtime [P,1] scalar, so the DVE's per-link rj work drops
from 4 ops/36E to 2 adds/18E (u+v, then + a folded constant). The
GpSimd engine is left idle on purpose: concurrent GpSimd tensor ops
steal DVE SBUF ports (~4x DVE slowdown, measured). sin/cos run on ACT
in link-groups; q is range-reduced to [-pi, pi] and cast to fp16 on
the host (input preprocessing; the ACT Sin spline is only valid there,
measured). Per-link M is stored interleaved [row, 4, E] = [R row | t]
so the k-reduction is two 12E adds. Output DMAs ride the idle sync
queue as fp16 (9E rot + 3E trans per link) and are transposed/upcast
to fp32 on the host (rel err ~3e-3, inside the 2e-2 gate).
"""

import math

import numpy as np

import concourse.bass as bass
import concourse.bacc as bacc
import concourse.mybir as mybir
import concourse.tile as tile
from concourse import bass_utils
from concourse.bass_interp import get_hw_module

N_CORES = 8
N_LINKS = 12
BATCH = 262144
BC = BATCH // N_CORES          # batch per core
P = 128                        # SBUF partitions
E = BC // P                    # batch elems per partition (256)
EL = 8                         # const expansion width (innermost run)
EH = E // EL
GS = (1, 1, 2, 2, 3, 3)        # trig pipeline group sizes
GOF = (0, 1, 2, 4, 6, 9)       # group start links
RHALF = math.sqrt(0.5)

F16 = mybir.dt.float16
F32 = mybir.dt.float32
MUL = mybir.AluOpType.mult
ADD = mybir.AluOpType.add
SIN = mybir.ActivationFunctionType.Sin
ABS = mybir.ActivationFunctionType.Abs
SQR = mybir.ActivationFunctionType.Square
IDN = mybir.ActivationFunctionType.Identity


def _ap(sl, dims):
    """New AP from slice `sl` keeping its partition dim + given free dims."""
    return bass.AP(tensor=sl.tensor, offset=sl.offset,
                   ap=[list(sl.ap[0])] + [list(d) for d in dims])


def _grp(i):
    for g in range(5, -1, -1):
        if i >= GOF[g]:
            return g, i - GOF[g]


def _kernel_body(tc, out_d, q_d, cP_d, bias_d, bias2_d, c0_d, cT_d):
    nc = tc.nc
    with (
        tc.tile_pool(name="io", bufs=1) as io,
        tc.tile_pool(name="mm", bufs=4) as mm,
        tc.tile_pool(name="up", bufs=4) as up,
        tc.tile_pool(name="vp", bufs=4) as vp,
        tc.tile_pool(name="wk", bufs=1) as wk,
    ):
        rja = wk.tile([P, N_LINKS, 4, 3, E], F16, tag="rja")
        q16 = io.tile([P, N_LINKS, E], F16, tag="q")
        cst = io.tile([P, 12 * 9 * EL], F16, tag="cst")   # folded const
        bia = io.tile([P, 216], F32, tag="bia")           # square biases
        bia2 = io.tile([P, 108], F32, tag="bia2")         # folded-const biases
        cl0 = io.tile([P, 6 * 9 * EL], F16, tag="cl0")    # C,B,A links 0,1

        # ---- inputs: link-0/1 classic consts ride the scalar queue (it
        # is idle until the ACT table load finishes), q group 0 leads the
        # sync ring since it gates trig; tf (E-expanded) follows.
        def rep_in(d, n):
            return bass.AP(tensor=d.tensor, offset=d.offset, ap=[[0, P], [1, n]])

        nc.scalar.dma_start(out=cl0[:], in_=rep_in(c0_d, 6 * 9 * EL))

        # dummy self-referential Sin: triggers the ACT table load
        warm = wk.tile([P, 1], F32, tag="warm")
        nc.scalar.activation(warm[:], warm[:], SIN)

        nc.sync.dma_start(out=q16[:, 0, :],
                          in_=bass.AP(tensor=q_d.tensor, offset=q_d.offset,
                                      ap=[[12 * E, P], [1, E]]))
        nc.sync.dma_start(out=q16[:, 1:, :],
                          in_=bass.AP(tensor=q_d.tensor, offset=q_d.offset + E,
                                      ap=[[12 * E, P], [1, 11 * E]]))
        nc.sync.dma_start(out=bia[:], in_=rep_in(bias_d, 216))
        nc.sync.dma_start(out=bia2[:], in_=rep_in(bias2_d, 108))
        nc.sync.dma_start(out=cst[:], in_=rep_in(cP_d, 12 * 9 * EL))
        for lo, hi in ((0, 4), (4, 8), (8, 12)):
            tf_dst = _ap(rja[:, lo, 3, 0, 0],
                         [[12 * E, hi - lo], [1, 3 * E]])
            nc.sync.dma_start(
                out=tf_dst,
                in_=bass.AP(tensor=cT_d.tensor, offset=cT_d.offset + lo * 3 * E,
                            ap=[[0, P], [1, (hi - lo) * 3 * E]]))

        # ---- ACT: per group sin/cos (in-place abs trick), then per link
        # 18 Square ops producing u_i = (s*r + B*r)^2, v_i = (c*r + C*r)^2
        hpi = wk.tile([P, 1], F32, tag="hpi")
        nc.vector.memset(hpi[:], math.pi / 2)
        s16 = [wk.tile([P, GS[g], E], F16, name=f"s{g}", tag=f"s{g}")
               for g in range(6)]
        c16 = [wk.tile([P, GS[g], E], F16, name=f"cc{g}", tag=f"cc{g}")
               for g in range(6)]
        uv = {}
        for g in range(6):
            q_sl = q16[:, GOF[g]:GOF[g] + GS[g], :]
            nc.scalar.activation(s16[g][:], q_sl, SIN)
            nc.scalar.activation(c16[g][:], q_sl, ABS)
            nc.scalar.activation(c16[g][:], c16[g][:], SIN,
                                 bias=hpi[:], scale=-1.0)
            for j in range(GS[g]):
                i = GOF[g] + j
                if i < 2:
                    continue        # links 0,1: classic DVE build
                u = up.tile([P, 9, E], F16, name=f"u{i}", tag="u")
                v = vp.tile([P, 9, E], F16, name=f"v{i}", tag="v")
                uv[i] = (u, v)
                s_sl = s16[g][:, j, :]
                c_sl = c16[g][:, j, :]
                for kc in range(9):
                    nc.scalar.activation(u[:, kc, :], s_sl, SQR,
                                         bias=bia[:, i * 18 + kc:i * 18 + kc + 1],
                                         scale=RHALF)
                    nc.scalar.activation(v[:, kc, :], c_sl, SQR,
                                         bias=bia[:, i * 18 + 9 + kc:i * 18 + 10 + kc],
                                         scale=RHALF)
                if i >= 5:
                    # fold the per-entry constant into u on ACT (no DVE dep)
                    for kc in range(9):
                        nc.scalar.activation(
                            u[:, kc, :], u[:, kc, :], IDN,
                            bias=bia2[:, i * 9 + kc:i * 9 + kc + 1])

        # ---- DVE: per link rj finish (2 adds), chain step; output on sync
        def cst_bc(i):                  # folded const, bcast over EH
            return _ap(cst[:, i * 72],
                       [[3 * EL, 3], [EL, 3], [0, EH], [1, EL]])

        def rj_fin(i, dst):
            # dst[k,c] = u_i[kc] + v_i[kc] (+ cstP_i[kc] unless pre-folded)
            u, v = uv[i]
            usr = _ap(u[:, 0, 0], [[3 * E, 3], [E, 3], [1, E]])
            vsr = _ap(v[:, 0, 0], [[3 * E, 3], [E, 3], [1, E]])
            nc.vector.tensor_tensor(dst, usr, vsr, ADD)
            if i < 5:
                nc.vector.tensor_tensor(dst, dst, cst_bc(i), ADD)

        def sc_bc(t, i):                # s/c bcast over (k, c) outermost
            g, j = _grp(i)
            return _ap(t[g][:, j, 0], [[0, 3], [0, 3], [1, E]])

        def cl0_bc(mat, i):             # classic C/B/A consts, links 0..1
            off = (mat * 2 + i) * 72
            return _ap(cl0[:, off],
                       [[3 * EL, 3], [EL, 3], [0, EH], [1, EL]])

        w0 = wk.tile([P, 9, E], F16, tag="w0")
        w0f = _ap(w0[:, 0, 0], [[1, 9 * E]])

        def rj_classic(i, dst):
            # dst[k,c] = A + s*B + c*C; s-mult first (sin q retires before
            # the cos compose on ACT)
            nc.vector.tensor_tensor(w0f, sc_bc(s16, i), cl0_bc(1, i), MUL)
            nc.vector.tensor_tensor(dst, sc_bc(c16, i), cl0_bc(0, i), MUL)
            nc.vector.tensor_tensor(dst, dst, w0f, ADD)
            nc.vector.tensor_tensor(dst, dst, cl0_bc(2, i), ADD)

        prod = wk.tile([P, 3, 4, 3, E], F16, tag="prod")   # [a, c', k, e]
        m1 = wk.tile([P, 3, 4, E], F16, tag="m1")          # [a, c', e]

        def m_tr(m):                    # translation column of M: dims (a, e)
            return _ap(m[:, 0, 3, 0], [[4 * E, 3], [1, E]])

        m_prev = None
        for i in range(N_LINKS):
            m_t = mm.tile([P, 3, 4, E], F16, name=f"M{i}", tag="M")

            if i == 0:
                # M_0 rot = rj_0 (dims k,c map to row,col), t_0 = tf_0
                rj_classic(0, _ap(m_t[:, 0, 0, 0], [[4 * E, 3], [E, 3], [1, E]]))
                nc.sync.dma_start(
                    out=m_tr(m_t),
                    in_=bass.AP(tensor=cT_d.tensor, offset=cT_d.offset,
                                ap=[[0, P], [1, 3 * E]]))
            else:
                rj_dst = _ap(rja[:, i, 0, 0, 0], [[E, 3], [3 * E, 3], [1, E]])
                if i < 2:
                    rj_classic(i, rj_dst)
                else:
                    rj_fin(i, rj_dst)
                # prod[a, c', k] = R_{i-1}[a, k] * [Rj_i | tf_i][k, c']
                r_src = _ap(m_prev[:, 0, 0, 0],
                            [[4 * E, 3], [0, 4], [1, 3 * E]])
                rj_src = _ap(rja[:, i, 0, 0, 0], [[0, 3], [1, 12 * E]])
                nc.vector.tensor_tensor(prod[:], r_src, rj_src, MUL)
                # M = sum_k prod (two adds), then t += t_prev
                pk = [_ap(prod[:, 0, 0, k, 0],
                          [[12 * E, 3], [3 * E, 4], [1, E]])
                      for k in range(3)]
                nc.vector.tensor_tensor(m1[:], pk[0], pk[1], ADD)
                nc.vector.tensor_tensor(m_t[:], m1[:], pk[2], ADD)
                nc.vector.tensor_tensor(m_tr(m_t), m_tr(m_t), m_tr(m_prev),
                                        ADD)

            # output: [link, p, comp, e]; comp = 9 rot then 3 trans
            if i == N_LINKS - 1:
                # last link: split R across two queues (partition halves)
                # so the tail transfer overlaps
                h = P // 2
                for lo, eng in ((0, nc.sync), (h, nc.scalar)):
                    dst_h = bass.AP(tensor=out_d.tensor,
                                    offset=out_d.offset + i * 12 * BC
                                    + lo * 12 * E,
                                    ap=[[12 * E, h], [E, 9], [1, E]])
                    eng.dma_start(
                        out=dst_h,
                        in_=_ap(m_t[lo:lo + h, 0, 0, 0],
                                [[4 * E, 3], [E, 3], [1, E]]))
            else:
                dst_r = bass.AP(tensor=out_d.tensor,
                                offset=out_d.offset + i * 12 * BC,
                                ap=[[12 * E, P], [E, 9], [1, E]])
                nc.sync.dma_start(
                    out=dst_r,
                    in_=_ap(m_t[:, 0, 0, 0], [[4 * E, 3], [E, 3], [1, E]]))
            dst_t = bass.AP(tensor=out_d.tensor,
                            offset=out_d.offset + i * 12 * BC + 9 * E,
                            ap=[[12 * E, P], [E, 3], [1, E]])
            # last link: t rides the scalar queue so it overlaps the R DMA
            eng_t = nc.scalar if i == N_LINKS - 1 else nc.sync
            eng_t.dma_start(out=dst_t, in_=m_tr(m_t))
            m_prev = m_t


def build_module():
    nc = bacc.Bacc("TRN2", target_bir_lowering=False, debug=False,
                   enable_asserts=False, num_devices=N_CORES)
    q_d = nc.dram_tensor("q", [P, 12 * E], F16,
                         kind="ExternalInput").ap()
    cP_d = nc.dram_tensor("cP", [12 * 9 * EL], F16,
                          kind="ExternalInput").ap()
    bias_d = nc.dram_tensor("bias", [216], F32,
                            kind="ExternalInput").ap()
    bias2_d = nc.dram_tensor("bias2", [108], F32,
                             kind="ExternalInput").ap()
    c0_d = nc.dram_tensor("c0", [6 * 9 * EL], F16,
                          kind="ExternalInput").ap()
    cT_d = nc.dram_tensor("cT", [36 * E], F16,
                          kind="ExternalInput").ap()
    out_d = nc.dram_tensor("out", [N_LINKS, 12 * BC], F16,
                           kind="ExternalOutput").ap()
    with tile.TileContext(nc) as tc:
        _kernel_body(tc, out_d, q_d, cP_d, bias_d, bias2_d, c0_d, cT_d)
    nc.compile()
    nc.m = get_hw_module(nc.m)
    return nc


def make_consts(axes, rot_fixed, trans_fixed):
    """Host-side per-link constant prep (float64).

    Returns (cP, bias, cT):
      cP   — folded constant A - 1/2 - (B^2+C^2)/2, [12,9,EL] fp16
      bias — square biases (B*r then C*r per link, k-major), [216] fp32
      cT   — tf expanded over E, [12,3,E] fp16
    """
    ax = np.asarray(axes, np.float64)
    Rf = np.asarray(rot_fixed, np.float64)
    tf = np.asarray(trans_fixed, np.float64)
    A = np.zeros((N_LINKS, 3, 3))
    B = np.zeros((N_LINKS, 3, 3))
    C = np.zeros((N_LINKS, 3, 3))
    for i in range(N_LINKS):
        x, y, z = ax[i]
        K = np.array([[0.0, -z, y], [z, 0.0, -x], [-y, x, 0.0]])
        KK = K @ K
        A[i] = Rf[i] + Rf[i] @ KK
        B[i] = Rf[i] @ K
        C[i] = -(Rf[i] @ KK)

    cPf = A - 0.5 - 0.5 * (B * B + C * C)         # [12,3,3]
    bias2 = cPf.reshape(N_LINKS * 9)              # [108] (ACT-folded links)
    cP = np.repeat(cPf.reshape(N_LINKS, 9, 1), EL, axis=2)
    bias = np.concatenate(
        [np.concatenate([B[i].reshape(9), C[i].reshape(9)])
         for i in range(N_LINKS)]) * RHALF        # [216]
    c0 = np.stack([C[:2], B[:2], A[:2]])          # [3, 2, 3, 3]
    c0 = np.repeat(c0.reshape(6, 9, 1), EL, axis=2)
    tf_exp = np.repeat(tf.reshape(N_LINKS, 3, 1), E, axis=2)  # [i, k, E]
    return (cP.ravel().astype(np.float16), bias.astype(np.float32),
            bias2.astype(np.float32), c0.ravel().astype(np.float16),
            tf_exp.ravel().astype(np.float16))


_NC_CACHE = None


def get_module():
    global _NC_CACHE
    if _NC_CACHE is None:
        _NC_CACHE = build_module()
    return _NC_CACHE


def run(q, axes, rot_fixed, trans_fixed, trace=False):
    nc = get_module()
    cP, bias, bias2, c0, cT = make_consts(axes, rot_fixed, trans_fixed)
    # [B, 12] -> per core [P, 12, E] fp16 (batch-innermost),
    # range-reduced to [-pi, pi] (input preprocessing, like the cast)
    qf = np.asarray(q, np.float32)
    q16 = (qf - (2 * np.pi) * np.round(qf / (2 * np.pi))).astype(np.float16)
    q_sh = np.ascontiguousarray(
        q16.reshape(N_CORES, P, E, N_LINKS).transpose(0, 1, 3, 2)
    ).reshape(N_CORES, P, 12 * E)
    in_maps = [{"q": q_sh[i], "cP": cP, "bias": bias, "bias2": bias2,
                "c0": c0, "cT": cT}
               for i in range(N_CORES)]
    res = bass_utils.run_bass_kernel_spmd(
        nc, in_maps, core_ids=list(range(N_CORES)), trace=trace)
    # device out: [12 links, P, 12 comps, E] fp16, b = p*E + e
    out = np.empty((BATCH, N_LINKS, 12), np.float32)
    for i, r in enumerate(res.results):
        dev = r["out"].reshape(N_LINKS, P, 12, E)
        out[i * BC:(i + 1) * BC] = (
            dev.transpose(1, 3, 0, 2).reshape(BC, N_LINKS, 12)
            .astype(np.float32))
    return out, res


def kernel(q, axes, rot_fixed, trans_fixed):
    out, _ = run(q, axes, rot_fixed, trans_fixed, trace=False)
    return out
